# revision 8
# baseline (speedup 1.0000x reference)
"""BERT multi-head attention on 8 Trainium2 NeuronCores, data-parallel over batch.

Problem: x[8,1024,768] fp32, 12 heads, qkv + masked softmax attention + out proj.
Each core handles one batch element end-to-end; host gathers the 8 outputs.

Per-core layout strategy (S=1024, D=768, H=12, Dh=64):
  - x is fed TRANSPOSED (xT [D,S]); q,k are produced transposed (qT/kT [D,S],
    stored bf16); scores are computed transposed (scoresT [k,q], f32 psum).
  - ctx runs in [q-partition, Dh-free] orientation: the exp'd scores p [k,q]
    (bf16) are the matmul STATIONARY and v (bf16, mask folded in, plus a
    masked ones-column) is the moving operand, so every ctx matmul is a
    fully-utilized 128x128x65 tile and the softmax denominator rides along
    as output column 64.
  - softmax normalization is free: a DVE reciprocal of the denominator column
    plus per-partition tensor_scalar_mul during the PSUM->SBUF evacuation.
  - ctx[q, (headA|headB)] 128x128 bf16 tiles are flipped to the [d-chunk, s]
    layout the output projection needs with the DMA xbar transpose
    (14ns/16x128-tile, zero PE cost).
  - k-bias is dropped entirely: softmax over k is invariant to the per-q
    constant (q+bq)@bk, so only the q-bias is applied.
  - ctx/out-proj matmuls are bf16 (same PE rate as f32r, halves SBUF/DMA);
    qkv projections and scores stay f32r-in / f32-accumulate.
  - PE fill discipline: qk projection chunks, v projection and the output
    projection are chopped into ~512-cycle closures and pumped into the
    scores/ctx instruction stream so the PE never waits on the Activation
    engine's exp (1038ns/tile vs 643ns/kt of attention matmuls).
  - PSUM: scores double-buffered (4 banks) + ctx accumulator (2 banks,
    [128, 8, 128] f32 so each 65-col group stays inside a bank) + a 2x1-bank
    fill pool shared by qk/v/out-proj = exactly 8 banks.
"""

import sys

import numpy as np

try:
    import concourse.bass as bass
except ImportError:  # pragma: no cover
    sys.path.insert(0, "/opt/trn_rl_repo")
    import concourse.bass as bass

from collections import deque
from contextlib import ExitStack

import concourse.tile as tile
from concourse import bacc, mybir
from concourse._compat import with_exitstack
from concourse.bass_utils import run_bass_kernel_spmd

F32 = mybir.dt.float32
F32R = mybir.dt.float32r
BF16 = mybir.dt.bfloat16
EXP = mybir.ActivationFunctionType.Exp

B, S, D, H, DH, P = 8, 1024, 768, 12, 64, 128
KC = D // P          # 6 contraction chunks of 128
NKT = S // P         # 8 k-tiles of 128
NP = H // 2          # 6 head pairs
SCALE = 1.0 / np.sqrt(DH)


@with_exitstack
def _emit(ctx: ExitStack, tc, out, xT, wqkv, bq, wout, beff, msk):
    nc = tc.nc

    const = ctx.enter_context(tc.tile_pool(name="const", bufs=1))
    persist = ctx.enter_context(tc.tile_pool(name="persist", bufs=1))
    wq_pool = ctx.enter_context(tc.tile_pool(name="wq", bufs=12))
    p_pool = ctx.enter_context(tc.tile_pool(name="p", bufs=16))
    small = ctx.enter_context(tc.tile_pool(name="small", bufs=2))
    ctxn_pool = ctx.enter_context(tc.tile_pool(name="ctxn", bufs=2))
    osb_pool = ctx.enter_context(tc.tile_pool(name="osb", bufs=2))

    # ------------- inputs / constants -------------
    # DMA emission order == service order on the DMA engines; load exactly
    # what the first compute needs first (wq 0/6 + x half 0), then the rest.
    wq_view = wqkv.rearrange("(c p) n -> p c n", p=P)  # [128, 6, 2304]
    xT_sb = persist.tile([P, KC, S], F32R)
    xT_view = xT.rearrange("(c p) s -> p c s", p=P).bitcast(F32R)
    wq_tiles = {}

    def load_wq(m):
        if m not in wq_tiles:
            t = wq_pool.tile([P, KC, P], F32R, tag="wq_t")
            nc.sync.dma_start(t[:], wq_view[:, :, m * P:(m + 1) * P]
                              .bitcast(F32R))
            wq_tiles[m] = t
        return wq_tiles[m]

    def load_wq_split(m):
        # chunk-granular load so the first matmuls can chase the DMA
        t = wq_pool.tile([P, KC, P], F32R, tag="wq_t")
        wq_tiles[m] = t
        for c in range(KC):
            nc.sync.dma_start(t[:, c], wq_view[:, c, m * P:(m + 1) * P]
                              .bitcast(F32R))

    load_wq_split(0)
    load_wq_split(KC)
    for c in range(KC):
        nc.sync.dma_start(xT_sb[:, c, 0:512], xT_view[:, c, 0:512])
    m_sb = const.tile([P, NKT], F32)
    nc.sync.dma_start(m_sb[:], msk.rearrange("(t p) -> p t", p=P))
    bq_sb = const.tile([P, KC], F32)
    nc.sync.dma_start(bq_sb[:], bq.rearrange("(c p) -> p c", p=P))
    wv_sb = persist.tile([P, KC, D], F32R)
    nc.sync.dma_start(wv_sb[:, :, 0:384],
                      wq_view[:, :, 2 * D:2 * D + 384].bitcast(F32R))
    for c in range(KC):
        nc.sync.dma_start(xT_sb[:, c, 512:1024], xT_view[:, c, 512:1024])
    nc.sync.dma_start(wv_sb[:, :, 384:768],
                      wq_view[:, :, 2 * D + 384:3 * D].bitcast(F32R))
    for m in (KC + 1, 1, KC + 2, 2, KC + 3, 3, KC + 4, 4, KC + 5, 5):
        load_wq(m)
    wo_sb = persist.tile([P, KC, D], BF16)
    nc.sync.dma_start(wo_sb[:], wout.rearrange("(c p) n -> p c n", p=P))
    beff_bc = const.tile([P, D], F32)
    nc.sync.dma_start(beff_bc[:], beff.partition_broadcast(P))
    ones_sb = const.tile([P, 6], F32)
    nc.vector.memset(ones_sb[:], 1.0)

    qkT_sb = persist.tile([P, 2 * KC, S], BF16)    # chunks 0..5 = qT, 6..11 = kT
    v_sb = persist.tile([P, NKT, H, DH + 1], BF16)  # masked v + masked ones col
    ctxT_sb = persist.tile([P, KC, S], BF16)        # [d-chunk, s], pair-major

    # ------------- psum pools -------------
    ps_s = ctx.enter_context(tc.tile_pool(name="ps_s", bufs=2, space="PSUM"))
    ps_ctx = ctx.enter_context(tc.tile_pool(name="ps_ctx", bufs=1, space="PSUM"))
    ps_fill = ctx.enter_context(tc.tile_pool(name="ps_fill", bufs=2, space="PSUM"))

    # ------------- fill work: qk projection halves, out projection ----------
    # Each closure is ~512 PE cycles; pumped between attention matmuls so the
    # PE always has slack-independent work while ACT exps catch up.
    fill_q = deque()

    def pump(n):
        for _ in range(n):
            if not fill_q:
                return
            fill_q.popleft()()

    def qk_half_closures(m, n):
        # 6 accumulation matmuls into a 1-bank fill tile + a DVE evac
        st = {}

        def chunk(c):
            def go():
                if c == 0:
                    st["t"] = ps_fill.tile([P, 512], F32, tag="fill", name="qkfill")
                wq_t = wq_tiles[m]
                nc.tensor.matmul(
                    st["t"][:, 0:512],
                    wq_t[:, c, :],
                    xT_sb[:, c, n * 512:(n + 1) * 512],
                    start=(c == 0), stop=(c == KC - 1))
                if c == KC - 1:
                    dst = qkT_sb[:, m, n * 512:(n + 1) * 512]
                    if m < KC:  # q gets its bias; k-bias is softmax-invariant
                        nc.vector.tensor_scalar_add(dst, st["t"][:, 0:512],
                                                    bq_sb[:, m:m + 1])
                    else:
                        nc.vector.tensor_copy(dst, st["t"][:, 0:512])
            return go
        return [chunk(c) for c in range(KC)]

    # ----- V projection, one s-chunk, one half (6 heads), masked + ones col --
    def emit_v_st(st_i, half):
        pv = ps_fill.tile([P, 512], F32, tag="fill")
        for c in range(KC):
            nc.tensor.matmul(
                pv[:, 0:384],
                xT_sb[:, c, st_i * P:(st_i + 1) * P],
                wv_sb[:, c, half * 384:(half + 1) * 384],
                start=(c == 0), stop=(c == KC - 1))
        hs = slice(half * 6, (half + 1) * 6)
        nc.vector.tensor_scalar_mul(
            v_sb[:, st_i, hs, 0:DH],
            pv[:, 0:384].rearrange("p (h d) -> p h d", h=6),
            m_sb[:, st_i:st_i + 1])
        nc.vector.tensor_scalar_mul(
            v_sb[:, st_i, hs, DH:DH + 1],
            ones_sb[:].unsqueeze(2),
            m_sb[:, st_i:st_i + 1])

    # ------------- out projection for one 128-row s-tile -------------
    def outproj_closures(qt):
        st = {}

        def mk(piece, c):
            def go():
                if c == 0:
                    st[piece] = ps_fill.tile([P, 512], F32, tag="fill", name="ofill")
                    if piece == 0:
                        st["o"] = osb_pool.tile([P, D], F32, tag="o", name="osb")
                nc.tensor.matmul(
                    st[piece][:, 0:384],
                    ctxT_sb[:, c, qt * P:(qt + 1) * P],
                    wo_sb[:, c, piece * 384:(piece + 1) * 384],
                    start=(c == 0), stop=(c == KC - 1))
                if c == KC - 1:
                    lo = piece * 384
                    nc.vector.tensor_add(st["o"][:, lo:lo + 384],
                                         st[piece][:, 0:384],
                                         beff_bc[:, lo:lo + 384])
                    if piece == 1:
                        nc.sync.dma_start(out[qt * P:(qt + 1) * P, :],
                                          st["o"][:])
            return go
        return [mk(piece, c) for piece in (0, 1) for c in range(KC)]

    # ------------- attention, pipelined one q-half deep -------------
    # A PSUM bank supports a single live accumulation group (a start=True
    # matmul marks the whole 2KB zero-region pending), so each ctx output
    # tile's 8-kt accumulation runs as one back-to-back group. The 8 groups
    # of q-half X are slotted one-per-kt into q-half X+1's score/exp stream;
    # the p tiles of X stay alive until then (p_pool bufs=16).
    def ctx_group(pair, qh, ctx_t, p_list, slot):
        # slot 0..7 -> groups A0,B0,A1,B1,... (banks alternate, so each bank
        # has at most one live group)
        j, half = slot // 2, slot % 2
        h = 2 * pair + half
        jj = 4 * half + j
        for kt in range(NKT):
            nc.tensor.matmul(
                ctx_t[:, jj, 0:DH + 1],
                p_list[kt][:, half * 512 + j * P:half * 512 + (j + 1) * P],
                v_sb[:, kt, h, :],
                start=(kt == 0), stop=(kt == NKT - 1),
                skip_group_check=True)

    def finish_qh(pair, qh, ctx_t):
        # normalization + evacuation + transpose: DVE + DMA only, zero PE
        r_sb = small.tile([P, NKT], F32, tag="r")
        nc.vector.reciprocal(r_sb[:, 0:4].unsqueeze(2),
                             ctx_t[:, 0:4, DH:DH + 1])
        nc.vector.reciprocal(r_sb[:, 4:8].unsqueeze(2),
                             ctx_t[:, 4:8, DH:DH + 1])
        for j in range(4):
            cn = ctxn_pool.tile([P, P], BF16, tag=f"cn{j % 2}", name="cn")
            nc.vector.tensor_scalar_mul(cn[:, 0:DH], ctx_t[:, j, 0:DH],
                                        r_sb[:, j:j + 1])
            nc.vector.tensor_scalar_mul(cn[:, DH:P], ctx_t[:, 4 + j, 0:DH],
                                        r_sb[:, 4 + j:4 + j + 1])
            cols = slice(qh * 512 + j * P, qh * 512 + (j + 1) * P)
            nc.sync.dma_start(ctxT_sb[:, pair, cols], cn[:], transpose=True)

    def attention(pair, qh, prev, v_half=None, ppk=2):
        # returns (pair, qh, p_list) to be consumed by the next call
        if isinstance(ppk, int):
            ppk = [ppk] * NKT
        qs = slice(qh * 512, (qh + 1) * 512)
        ctx_t = None
        p_list = []
        for kt in range(NKT):
            s_ps = ps_s.tile([P, 1024], F32, tag="s_ps")
            nc.tensor.matmul(
                s_ps[:, 0:512],
                qkT_sb[0:DH, KC + pair, kt * P:(kt + 1) * P],
                qkT_sb[0:DH, pair, qs],
                start=True, stop=True, tile_position=(0, 0))
            nc.tensor.matmul(
                s_ps[:, 512:1024],
                qkT_sb[DH:P, KC + pair, kt * P:(kt + 1) * P],
                qkT_sb[DH:P, pair, qs],
                start=True, stop=True, tile_position=(DH, 0))
            p_t = p_pool.tile([P, 1024], BF16)
            nc.scalar.activation(p_t[:], s_ps[:], EXP, bias=0.0, scale=SCALE)
            p_list.append(p_t)
            if v_half is not None:
                emit_v_st(kt, v_half)
            if prev is not None:
                # groups packed into slots kt0..kt6 (two at kt0) so the
                # finish chain at kt7 gives DVE a head start on the evacs
                if ctx_t is None:
                    ctx_t = ps_ctx.tile([P, NKT, P], F32, tag="ctx")
                if kt == 0:
                    ctx_group(prev[0], prev[1], ctx_t, prev[2], 0)
                    ctx_group(prev[0], prev[1], ctx_t, prev[2], 1)
                elif kt < NKT - 1:
                    ctx_group(prev[0], prev[1], ctx_t, prev[2], kt + 1)
                else:
                    finish_qh(prev[0], prev[1], ctx_t)
            pump(ppk[kt])
        return (pair, qh, p_list)

    def drain_last(prev):
        ctx_t = ps_ctx.tile([P, NKT, P], F32, tag="ctx")
        for slot in range(NKT):
            pump(2)
            ctx_group(prev[0], prev[1], ctx_t, prev[2], slot)
        finish_qh(prev[0], prev[1], ctx_t)

    # ------------- phase structure -------------
    with nc.allow_low_precision(reason="bf16 stores are within the 2e-2 gate"):
        # pre-attention: q pair0 half0 + k pair0 both halves, chasing the DMA
        tq = ps_fill.tile([P, 512], F32, tag="fill")
        tk = ps_fill.tile([P, 512], F32, tag="fill")
        for c in range(KC):
            nc.tensor.matmul(tq[:, 0:512], wq_tiles[0][:, c, :],
                             xT_sb[:, c, 0:512],
                             start=(c == 0), stop=(c == KC - 1))
            nc.tensor.matmul(tk[:, 0:512], wq_tiles[KC][:, c, :],
                             xT_sb[:, c, 0:512],
                             start=(c == 0), stop=(c == KC - 1))
        nc.vector.tensor_scalar_add(qkT_sb[:, 0, 0:512], tq[:, 0:512],
                                    bq_sb[:, 0:1])
        nc.vector.tensor_copy(qkT_sb[:, KC, 0:512], tk[:, 0:512])
        for fn in qk_half_closures(KC, 1):
            fn()

        # sweep order: pair-major, with (5,0) hoisted before (4,0) so all six
        # qh0 ctxT halves are transposed before (5,1) — the out projection for
        # s-tiles 0..3 then overlaps the final score/exp sweep.
        # fill schedule constraints: q(p,1) before the (p,1) sweep; k(p+1,*)
        # and q(p+1,0) before (p+1,0) starts (k half1 only by its kt=4).
        # v-interleave sweeps pump little (the v projection owns the fill
        # psum pool there and already over-fills the PE); q-half prereqs of
        # the NEXT item run as compact post-blocks after those sweeps.
        sweep = [(0, 0), (0, 1), (1, 0), (1, 1), (2, 0), (2, 1),
                 (3, 0), (3, 1), (5, 0), (4, 0), (4, 1), (5, 1)]
        plan = {
            (0, 0): ([(KC + 1, 0)], [(0, 1)], 1),             # v half0 inline
            (0, 1): ([(KC + 1, 1), (1, 0), (KC + 2, 0)], [], 3),
            (1, 0): ([], [(1, 1)], 0),                        # v half1 inline
            (1, 1): ([(KC + 2, 1), (2, 0), (KC + 3, 0)], [], 3),
            (2, 0): ([(KC + 3, 1), (2, 1)], [], 2),
            (2, 1): ([(3, 0), (KC + 5, 0)], [], 2),
            (3, 0): ([(KC + 5, 1), (3, 1)], [], 2),
            (3, 1): ([(5, 0), (KC + 4, 0)], [], 2),
            (5, 0): ([(KC + 4, 1), (4, 0)], [], 2),
            (4, 0): ([(4, 1), (5, 1)], [], 2),
            (4, 1): ([], [], 1),
            (5, 1): (None, [], [0, 0, 7, 7, 7, 7, 7, 7]),     # outproj qt 0-3
        }
        prev = None
        for pair, qh in sweep:
            pumped, post, ppk = plan[(pair, qh)]
            if pumped is None:
                for qt in range(4):
                    fill_q.extend(outproj_closures(qt))
            else:
                for key in pumped:
                    fill_q.extend(qk_half_closures(*key))
            v_half = 0 if (pair, qh) == (0, 0) else \
                1 if (pair, qh) == (1, 0) else None
            prev = attention(pair, qh, prev, v_half=v_half, ppk=ppk)
            for key in post:
                for fn in qk_half_closures(*key):
                    fn()
        drain_last(prev)
        pump(len(fill_q))
        for qt in range(4, NKT):
            for fn in outproj_closures(qt):
                fn()


_CACHE = {}


def _build():
    if "nc" in _CACHE:
        return _CACHE["nc"]
    nc = bacc.Bacc("TRN2", target_bir_lowering=False, debug=False,
                   num_devices=B)
    xT = nc.dram_tensor("xt", [D, S], F32, kind="ExternalInput").ap()
    wqkv = nc.dram_tensor("wqkv", [D, 3 * D], F32, kind="ExternalInput").ap()
    bq = nc.dram_tensor("bq", [D], F32, kind="ExternalInput").ap()
    wout = nc.dram_tensor("wout", [D, D], BF16, kind="ExternalInput").ap()
    beff = nc.dram_tensor("beff", [D], F32, kind="ExternalInput").ap()
    msk = nc.dram_tensor("msk", [S], F32, kind="ExternalInput").ap()
    out = nc.dram_tensor("out", [S, D], F32, kind="ExternalOutput").ap()
    with tile.TileContext(nc) as tc:
        _emit(tc, out, xT, wqkv, bq, wout, beff, msk)
    nc.compile()
    _CACHE["nc"] = nc
    return nc


def _in_maps(x, mask, W_qkv, b_qkv, W_out, b_out):
    import ml_dtypes
    xT = np.ascontiguousarray(np.transpose(
        np.asarray(x, dtype=np.float32), (0, 2, 1)))          # [8, 768, 1024]
    m = np.asarray(mask).reshape(B, S).astype(np.float32)
    bq = np.ascontiguousarray(np.asarray(b_qkv, np.float32)[:D])
    beff = (np.asarray(b_qkv, np.float64)[2 * D:] @ np.asarray(W_out, np.float64)
            + np.asarray(b_out, np.float64)).astype(np.float32)
    wqkv = np.ascontiguousarray(np.asarray(W_qkv, np.float32))
    wout = np.asarray(W_out, np.float32).astype(ml_dtypes.bfloat16)
    return [
        {"xt": xT[b], "msk": m[b], "wqkv": wqkv, "bq": bq,
         "wout": wout, "beff": beff}
        for b in range(B)
    ]


def kernel(x, mask, W_qkv, b_qkv, W_out, b_out):
    nc = _build()
    maps = _in_maps(x, mask, W_qkv, b_qkv, W_out, b_out)
    res = run_bass_kernel_spmd(nc, maps, list(range(B))).results
    out = np.stack([res[b]["out"] for b in range(B)]).astype(np.float32)
    return out


# revision 28
# speedup vs baseline: 1.1727x; 1.1727x over previous
"""BERT multi-head attention on 8 Trainium2 NeuronCores, data-parallel over batch.

Problem: x[8,1024,768] fp32, 12 heads, qkv + masked softmax attention + out proj.
Each core handles one batch element end-to-end; host gathers the 8 outputs.

Per-core layout strategy (S=1024, D=768, H=12, Dh=64):
  - all matmuls are bf16 x bf16 with f32 PSUM accumulation (same PE rate as
    f32r, half the DMA/SBUF): x and the weights are host-cast to bf16.
  - x is fed TRANSPOSED (xT [D,S]); q,k are produced transposed (qT/kT [D,S]);
    scores are computed transposed (scoresT [k,q], f32 psum, 2 heads packed
    per 128-row PE pass via tile_position).
  - ctx runs in [q-partition, Dh-free] orientation: the exp'd scores p [k,q]
    (bf16) are the matmul STATIONARY and v (bf16, mask folded in, plus a
    masked ones-column) is the moving operand, so every ctx matmul is a
    fully-utilized 128x128x65 tile and the softmax denominator rides along
    as output column 64. A PSUM bank holds one live accumulation group
    (start=True marks the whole 2KB zero-region), so each ctx tile's 8-kt
    accumulation runs as one back-to-back group; the 8 groups of q-half X
    are slotted into q-half X+1's score/exp stream (p tiles persist, 16 bufs).
  - softmax normalization is free: a DVE reciprocal of the denominator column
    plus per-partition tensor_scalar_mul during the PSUM->SBUF evacuation.
  - ctx[q, headA|headB] tiles are flipped to the [d-chunk, s] layout the
    output projection needs by ONE DMA xbar block-transpose per q-half
    ([128,512] -> 4x[128,128] blocks, 3D out AP; zero PE cost).
  - k-bias is dropped entirely: softmax over k is invariant to the per-q
    constant (q+bq)@bk, so only the q-bias is applied.
  - PE fill discipline: qk projection chunks and the output projection are
    chopped into ~512-cycle closures and pumped between attention matmuls so
    the PE never waits on the Activation engine's exp (1038ns/tile vs 643ns
    of attention matmuls per kt). The sweep is qh0-front so all six ctxT
    qh0 halves finish early and the s-tile 0..3 output projection overlaps
    the last three score sweeps.
  - PSUM: scores double-buffered (4 banks) + ctx accumulator (2 banks,
    [128, 8, 128] f32 so each 65-col group stays inside a bank) + a 2x1-bank
    fill pool shared by qk/v/out-proj = exactly 8 banks.
"""

import sys

import numpy as np

try:
    import concourse.bass as bass
except ImportError:  # pragma: no cover
    sys.path.insert(0, "/opt/trn_rl_repo")
    import concourse.bass as bass

from collections import deque
from contextlib import ExitStack

import concourse.tile as tile
from concourse import bacc, mybir
from concourse._compat import with_exitstack
from concourse.bass_utils import run_bass_kernel_spmd

F32 = mybir.dt.float32
BF16 = mybir.dt.bfloat16
EXP = mybir.ActivationFunctionType.Exp
COPY = mybir.ActivationFunctionType.Copy

B, S, D, H, DH, P = 8, 1024, 768, 12, 64, 128
KC = D // P          # 6 contraction chunks of 128
NKT = S // P         # 8 k-tiles of 128
NP = H // 2          # 6 head pairs
SCALE = 1.0 / np.sqrt(DH)


@with_exitstack
def _emit(ctx: ExitStack, tc, out, xT, wqkv, wout, beff, msk):
    nc = tc.nc

    const = ctx.enter_context(tc.tile_pool(name="const", bufs=1))
    persist = ctx.enter_context(tc.tile_pool(name="persist", bufs=1))
    p_pool = ctx.enter_context(tc.tile_pool(name="p", bufs=16))
    small = ctx.enter_context(tc.tile_pool(name="small", bufs=2))
    ctxn_pool = ctx.enter_context(tc.tile_pool(name="ctxn", bufs=2))
    osb_pool = ctx.enter_context(tc.tile_pool(name="osb", bufs=4))

    # ------------- inputs / constants -------------
    # DMA emission order == service order; the first score matmuls need
    # wq chunk 0, wq chunk 6 and x half 0, so those three go first.
    # wqkv arrives host-permuted as [q0|k0|q1..q5|k1..k5|v]: the first DMA
    # grabs both chunk-0 stationaries in one contiguous 256-col slice.
    wq_view = wqkv.rearrange("(c p) n -> p c n", p=P)   # [128, 6, 2304] bf16
    xT_view = xT.rearrange("(c p) s -> p c s", p=P)     # [128, 6, 1024] bf16
    wqk_sb = persist.tile([P, KC, 2 * D], BF16)         # permuted q|k layout
    xT_sb = persist.tile([P, KC, S], BF16)
    misc_sb = const.tile([P, NKT + KC], F32)            # mask cols 0:8, bq 8:14
    for h in range(3):  # chunk-pair pieces so the first matmuls can chase
        cs = slice(2 * h, 2 * h + 2)
        nc.sync.dma_start(wqk_sb[:, cs, 0:2 * P], wq_view[:, cs, 0:2 * P])
        nc.sync.dma_start(xT_sb[:, cs, 0:512], xT_view[:, cs, 0:512])
        if h == 0:
            nc.sync.dma_start(misc_sb[:], msk.rearrange("(t p) -> p t", p=P))
    wv_sb = persist.tile([P, KC, D], BF16)
    nc.sync.dma_start(wqk_sb[:, :, 7 * P:8 * P], wq_view[:, :, 7 * P:8 * P])
    nc.sync.dma_start(xT_sb[:, :, 512:1024], xT_view[:, :, 512:1024])
    nc.sync.dma_start(wv_sb[:, :, 0:384], wq_view[:, :, 2 * D:2 * D + 384])
    nc.sync.dma_start(wv_sb[:, :, 384:768], wq_view[:, :, 2 * D + 384:3 * D])
    nc.sync.dma_start(wqk_sb[:, :, 2 * P:7 * P], wq_view[:, :, 2 * P:7 * P])
    nc.sync.dma_start(wqk_sb[:, :, 8 * P:12 * P], wq_view[:, :, 8 * P:12 * P])
    wo_sb = persist.tile([P, KC, D], BF16)
    nc.sync.dma_start(wo_sb[:], wout.rearrange("(c p) n -> p c n", p=P))
    beff_bc = const.tile([P, D], F32)
    nc.sync.dma_start(beff_bc[:], beff.partition_broadcast(P))
    ones_sb = const.tile([P, 6], F32)
    nc.vector.memset(ones_sb[:], 1.0)

    _wq_off = {m: (0 if m == 0 else P if m == KC else
                   (1 + m) * P if m < KC else (m - KC + 6) * P)
               for m in range(2 * KC)}

    def wq_t(m, c):  # stationary slice for qk chunk m in the permuted layout
        o = _wq_off[m]
        return wqk_sb[:, c, o:o + P]

    qkT_sb = persist.tile([P, 2 * KC, S], BF16)    # chunks 0..5 = qT, 6..11 = kT
    v_sb = persist.tile([P, NKT, H, DH + 1], BF16)  # masked v + masked ones col
    ctxT0 = persist.tile([P, KC, 512], BF16)   # [d-chunk, s 0:512]
    ctxT1 = persist.tile([P, KC, 512], BF16)   # [d-chunk, s 512:1024]
    ctxT3d = [ctxT0[:].rearrange("p c (j q) -> p c j q", q=P),
              ctxT1[:].rearrange("p c (j q) -> p c j q", q=P)]

    def ctxT(c, qt):  # [128, 128] stationary block for s-tile qt
        t = ctxT0 if qt < 4 else ctxT1
        return t[:, c, (qt % 4) * P:(qt % 4 + 1) * P]

    # ------------- psum pools -------------
    ps_s = ctx.enter_context(tc.tile_pool(name="ps_s", bufs=2, space="PSUM"))
    ps_ctx = ctx.enter_context(tc.tile_pool(name="ps_ctx", bufs=1, space="PSUM"))
    ps_fill = ctx.enter_context(tc.tile_pool(name="ps_fill", bufs=2, space="PSUM"))

    # ------------- fill work: qk projection halves, out projection ----------
    # Each closure is ~512 PE cycles; pumped between attention matmuls so the
    # PE always has slack-independent work while ACT exps catch up.
    fill_q = deque()

    def pump(n):
        for _ in range(n):
            if not fill_q:
                return
            fill_q.popleft()()

    def qk_half_closures(m, n):
        # 6 accumulation matmuls into a 1-bank fill tile + a DVE evac
        st = {}

        def chunk(c):
            def go():
                if c == 0:
                    st["t"] = ps_fill.tile([P, 512], F32, tag="fill",
                                           name="qkfill")
                nc.tensor.matmul(
                    st["t"][:, 0:512],
                    wq_t(m, c),
                    xT_sb[:, c, n * 512:(n + 1) * 512],
                    start=(c == 0), stop=(c == KC - 1))
                if c == KC - 1:
                    dst = qkT_sb[:, m, n * 512:(n + 1) * 512]
                    if m < KC:  # q gets its bias; k-bias is softmax-invariant
                        nc.vector.tensor_scalar_add(dst, st["t"][:, 0:512],
                                                    misc_sb[:, NKT + m:NKT + m + 1])
                    else:
                        nc.vector.tensor_copy(dst, st["t"][:, 0:512])
            return go
        return [chunk(c) for c in range(KC)]

    # ----- V projection, one s-chunk, one half (6 heads), masked + ones col --
    def emit_v_st(st_i, half):
        pv = ps_fill.tile([P, 512], F32, tag="fill")
        for c in range(KC):
            nc.tensor.matmul(
                pv[:, 0:384],
                xT_sb[:, c, st_i * P:(st_i + 1) * P],
                wv_sb[:, c, half * 384:(half + 1) * 384],
                start=(c == 0), stop=(c == KC - 1))
        hs = slice(half * 6, (half + 1) * 6)
        nc.scalar.activation(
            v_sb[:, st_i, hs, 0:DH],
            pv[:, 0:384].rearrange("p (h d) -> p h d", h=6),
            COPY, scale=misc_sb[:, st_i:st_i + 1])
        nc.gpsimd.tensor_scalar_mul(
            v_sb[:, st_i, hs, DH:DH + 1],
            ones_sb[:].unsqueeze(2),
            misc_sb[:, st_i:st_i + 1])

    # ------------- out projection for one 128-row s-tile -------------
    def outproj_closures(qt):
        st = {}

        def mk(piece, c):
            def go():
                if c == 0:
                    st[piece] = ps_fill.tile([P, 512], F32, tag="fill",
                                             name="ofill")
                    if piece == 0:
                        st["o"] = osb_pool.tile([P, D], BF16, tag="o",
                                                name="osb")
                nc.tensor.matmul(
                    st[piece][:, 0:384],
                    ctxT(c, qt),
                    wo_sb[:, c, piece * 384:(piece + 1) * 384],
                    start=(c == 0), stop=(c >= KC - 2),
                    skip_group_check=(c == KC - 1))
                lo = piece * 384
                if c == KC - 2:
                    # fold the bias into the open psum group so the final
                    # evacuation is a cheap 2x-mode copy
                    nc.vector.tensor_add(st[piece][:, 0:384],
                                         st[piece][:, 0:384],
                                         beff_bc[:, lo:lo + 384])
                if c == KC - 1:
                    nc.vector.tensor_copy(st["o"][:, lo:lo + 384],
                                          st[piece][:, 0:384])
                    if piece == 1:
                        nc.sync.dma_start(out[qt * P:(qt + 1) * P, :],
                                          st["o"][:])
            return go
        order = ([(0, c) for c in range(KC - 1)]
                 + [(1, c) for c in range(KC - 1)]
                 + [(0, KC - 1), (1, KC - 1)])
        return [mk(piece, c) for piece, c in order]

    # ------------- attention, pipelined one q-half deep -------------
    def ctx_group(pair, ctx_t, p_list, slot):
        # slot 0..7 -> groups A0,B0,A1,B1,... (banks alternate, so each bank
        # has at most one live accumulation group)
        j, half = slot // 2, slot % 2
        h = 2 * pair + half
        jj = 4 * half + j
        for kt in range(NKT):
            nc.tensor.matmul(
                ctx_t[:, jj, 0:DH + 1],
                p_list[kt][:, half * 512 + j * P:half * 512 + (j + 1) * P],
                v_sb[:, kt, h, :],
                start=(kt == 0), stop=(kt == NKT - 1),
                skip_group_check=True)

    def finish_qh(pair, qh, ctx_t, fast=False):
        # normalization + evacuation + transpose: one strided DVE reciprocal,
        # 8 evac-muls, two xbar half-transposes. fast=True splits the muls
        # across DVE and Pool for the tail-critical last q-half.
        r_sb = small.tile([P, NKT], F32, tag="r")
        nc.vector.reciprocal(r_sb[:].unsqueeze(2), ctx_t[:, :, DH:DH + 1])
        cn = ctxn_pool.tile([P, 512], BF16, tag="cn", name="cn")

        def mul(j, eng):
            if eng is nc.scalar:
                nc.scalar.activation(cn[:, j * P:j * P + DH],
                                     ctx_t[:, j, 0:DH], COPY,
                                     scale=r_sb[:, j:j + 1])
                nc.scalar.activation(cn[:, j * P + DH:(j + 1) * P],
                                     ctx_t[:, 4 + j, 0:DH], COPY,
                                     scale=r_sb[:, 4 + j:4 + j + 1])
                return
            eng.tensor_scalar_mul(cn[:, j * P:j * P + DH],
                                  ctx_t[:, j, 0:DH], r_sb[:, j:j + 1])
            eng.tensor_scalar_mul(cn[:, j * P + DH:(j + 1) * P],
                                  ctx_t[:, 4 + j, 0:DH],
                                  r_sb[:, 4 + j:4 + j + 1])

        e1 = nc.scalar if fast else nc.vector
        mul(0, nc.vector)
        mul(1, e1)
        nc.sync.dma_start(ctxT3d[qh][:, pair, 0:2, :],
                          cn[:, 0:256], transpose=True)
        mul(2, nc.vector)
        mul(3, e1)
        nc.sync.dma_start(ctxT3d[qh][:, pair, 2:4, :],
                          cn[:, 256:512], transpose=True)

    def attention(pair, qh, prev, v_half=None, ppk=2, pre_kt=None):
        # returns (pair, qh, p_list) to be consumed by the next call
        if isinstance(ppk, int):
            ppk = [ppk] * NKT
        pre_kt = pre_kt or {}
        qs = slice(qh * 512, (qh + 1) * 512)
        ctx_t = None
        p_list = []
        for kt in range(NKT):
            for fn in pre_kt.get(kt, ()):
                fn()
            s_ps = ps_s.tile([P, 1024], F32, tag="s_ps")
            nc.tensor.matmul(
                s_ps[:, 0:512],
                qkT_sb[0:DH, KC + pair, kt * P:(kt + 1) * P],
                qkT_sb[0:DH, pair, qs],
                start=True, stop=True, tile_position=(0, 0))
            nc.tensor.matmul(
                s_ps[:, 512:1024],
                qkT_sb[DH:P, KC + pair, kt * P:(kt + 1) * P],
                qkT_sb[DH:P, pair, qs],
                start=True, stop=True, tile_position=(DH, 0))
            p_t = p_pool.tile([P, 1024], BF16)
            nc.scalar.activation(p_t[:], s_ps[:], EXP, bias=0.0, scale=SCALE)
            p_list.append(p_t)
            if v_half is not None and kt > 3:
                emit_v_st(kt - 4, v_half)
            if prev is not None:
                # ctx groups sit at kts 2..5 (two per kt, banks alternating):
                # late enough that the previous item's last exp has landed,
                # early enough that the finish chain at kt6 has 4 kts of
                # slack before the next item's first ctx write
                if ctx_t is None:
                    ctx_t = ps_ctx.tile([P, NKT, P], F32, tag="ctx")
                for slot in ((), (), (0, 1), (2, 3), (4, 5), (6, 7), (), ())[kt]:
                    ctx_group(prev[0], ctx_t, prev[2], slot)
                if kt == 6:
                    finish_qh(prev[0], prev[1], ctx_t)
            pump(ppk[kt])
        return (pair, qh, p_list)

    def drain_last(prev):
        ctx_t = ps_ctx.tile([P, NKT, P], F32, tag="ctx")
        for slot in range(NKT):
            pump(2)
            ctx_group(prev[0], ctx_t, prev[2], slot)
        finish_qh(prev[0], prev[1], ctx_t, fast=True)

    # ------------- phase structure -------------
    with nc.allow_low_precision(reason="bf16 stores are within the 2e-2 gate"):
        # pre-attention: q(0, half0) + k(6, half0), interleaved per chunk so
        # the matmuls chase the x DMA pieces
        tq = ps_fill.tile([P, 512], F32, tag="fill")
        tk = ps_fill.tile([P, 512], F32, tag="fill")
        for c in range(KC):
            nc.tensor.matmul(tq[:, 0:512], wq_t(0, c), xT_sb[:, c, 0:512],
                             start=(c == 0), stop=(c == KC - 1))
            nc.tensor.matmul(tk[:, 0:512], wq_t(KC, c), xT_sb[:, c, 0:512],
                             start=(c == 0), stop=(c == KC - 1))
        nc.vector.tensor_scalar_add(qkT_sb[:, 0, 0:512], tq[:, 0:512],
                                    misc_sb[:, NKT:NKT + 1])
        nc.vector.tensor_copy(qkT_sb[:, KC, 0:512], tk[:, 0:512])

        # sweep: qh0-front so every ctxT qh0 half is transposed by item 9 and
        # the s-tile 0..3 out projection pumps through items 10-12.
        # fill deadlines (emission order, in-order PE): a half must be fully
        # emitted before the first score matmul that reads it.
        k61 = qk_half_closures(KC, 1)   # k(6) half1: inline before kt4, needs
        #                                 x half1 which lands mid-item-1
        P18 = [3, 3, 1, 1, 1, 1, 4, 4]
        P12 = [2, 2, 1, 1, 1, 1, 2, 2]
        P16 = [3, 3, 1, 1, 1, 1, 3, 3]
        sweep = [
            # (pair, qh, v_half, pumped_halves, ppk)
            (0, 0, 0, [(KC + 1, 0)], [2, 2, 1, 1, 0, 0, 0, 0]),
            (0, 1, None, [(KC + 1, 1), (1, 0), (KC + 2, 0)], P18),
            (1, 0, 1, [], 0),
            (1, 1, None, [(KC + 2, 1), (2, 0), (KC + 3, 0)], P18),
            (2, 0, None, [(KC + 3, 1), (3, 0), (KC + 4, 0)], P18),
            (3, 0, None, [(KC + 4, 1), (4, 0), (KC + 5, 0)], P18),
            (4, 0, None, [(KC + 5, 1), (5, 0)], P12),
            (5, 0, None, [(2, 1), (3, 1)], P12),
            (2, 1, None, [(4, 1), (5, 1)], P12),
            (3, 1, "op", [], P16),
            (4, 1, None, [], P16),
            (5, 1, None, [], P16),
        ]
        posts = {(0, 0): [(0, 1)], (1, 0): [(1, 1)]}
        prev = None
        for i, (pair, qh, vh, pumped, ppk) in enumerate(sweep):
            if vh == "op":
                vh = None
                for qt in range(4):
                    fill_q.extend(outproj_closures(qt))
            for key in pumped:
                fill_q.extend(qk_half_closures(*key))
            pre = {}
            if i == 0:
                # k(6,1) emitted compactly before kt4 (its x-half-1 input
                # lands mid-item); v s-chunks run four kts late (the wv and
                # x DMAs land mid-item), spilling into the next item's pre
                pre = {4: k61}
            if i in (1, 3):
                vh_prev = 0 if i == 1 else 1
                pre = {0: [lambda h=vh_prev: emit_v_st(4, h),
                           lambda h=vh_prev: emit_v_st(5, h)],
                       1: [lambda h=vh_prev: emit_v_st(6, h),
                           lambda h=vh_prev: emit_v_st(7, h)]}
            prev = attention(pair, qh, prev, v_half=vh, ppk=ppk, pre_kt=pre)
            for key in posts.get((pair, qh), ()):
                for fn in qk_half_closures(*key):
                    fn()
        drain_last(prev)
        pump(len(fill_q))
        pieces = []   # (qt, piece, psum ap)
        for qt in (4, 5):
            t = ps_s.tile([P, 1024], F32, tag="s_ps", name="ost")
            pieces += [(qt, 0, t[:, 0:384]), (qt, 1, t[:, 512:896])]
        tc6 = ps_ctx.tile([P, NKT, P], F32, tag="ctx", name="oct")
        t6 = tc6[:].rearrange("p j q -> p (j q)")
        pieces += [(6, 0, t6[:, 0:384]), (6, 1, t6[:, 512:896])]
        for qt in (7,):
            ta = ps_fill.tile([P, 512], F32, tag="fill", name="ofa")
            tb = ps_fill.tile([P, 512], F32, tag="fill", name="ofb")
            pieces += [(qt, 0, ta[:, 0:384]), (qt, 1, tb[:, 0:384])]
        for qt, piece, ap in pieces:        # pairs 0..4 stream immediately
            for c in range(KC - 1):
                nc.tensor.matmul(ap, ctxT(c, qt),
                                 wo_sb[:, c, piece * 384:(piece + 1) * 384],
                                 start=(c == 0), stop=(c == KC - 2))
            nc.vector.tensor_add(ap, ap, beff_bc[:, piece * 384:
                                                 piece * 384 + 384])
        osbs = {}
        for qt, piece, ap in pieces:        # pair-5 chunk gated on transpose
            nc.tensor.matmul(ap, ctxT(KC - 1, qt),
                             wo_sb[:, KC - 1, piece * 384:(piece + 1) * 384],
                             start=False, stop=True, skip_group_check=True)
            if piece == 0:
                osbs[qt] = osb_pool.tile([P, D], BF16, tag="o", name="osb")
            lo = piece * 384
            if piece == 0:
                nc.vector.tensor_copy(osbs[qt][:, lo:lo + 384], ap)
            else:
                nc.scalar.activation(osbs[qt][:, lo:lo + 384], ap, COPY)
            if piece == 1:
                nc.sync.dma_start(out[qt * P:(qt + 1) * P, :], osbs[qt][:])


_CACHE = {}


def _build():
    if "nc" in _CACHE:
        return _CACHE["nc"]
    nc = bacc.Bacc("TRN2", target_bir_lowering=False, debug=False,
                   num_devices=B)
    xT = nc.dram_tensor("xt", [D, S], BF16, kind="ExternalInput").ap()
    wqkv = nc.dram_tensor("wqkv", [D, 3 * D], BF16, kind="ExternalInput").ap()
    wout = nc.dram_tensor("wout", [D, D], BF16, kind="ExternalInput").ap()
    beff = nc.dram_tensor("beff", [D], F32, kind="ExternalInput").ap()
    msk = nc.dram_tensor("msk", [P * (NKT + KC)], F32,
                         kind="ExternalInput").ap()
    out = nc.dram_tensor("out", [S, D], BF16, kind="ExternalOutput").ap()
    with tile.TileContext(nc) as tc:
        _emit(tc, out, xT, wqkv, wout, beff, msk)
    nc.compile()
    _CACHE["nc"] = nc
    return nc


def _in_maps(x, mask, W_qkv, b_qkv, W_out, b_out):
    import ml_dtypes
    xT = np.ascontiguousarray(np.transpose(
        np.asarray(x, dtype=np.float32), (0, 2, 1))).astype(ml_dtypes.bfloat16)
    m = np.asarray(mask).reshape(B, S).astype(np.float32)
    bq = np.asarray(b_qkv, np.float32)[:D]
    beff = (np.asarray(b_qkv, np.float64)[2 * D:] @ np.asarray(W_out, np.float64)
            + np.asarray(b_out, np.float64)).astype(np.float32)
    w = np.asarray(W_qkv, np.float32)
    # column permutation [q0|k0|q1..q5|k1..k5|v] so the kernel's first weight
    # DMA is one contiguous slice covering both chunk-0 stationaries
    wqkv = np.concatenate(
        [w[:, 0:P], w[:, D:D + P], w[:, P:D], w[:, D + P:2 * D],
         w[:, 2 * D:]], axis=1).astype(ml_dtypes.bfloat16)
    wout = np.asarray(W_out, np.float32).astype(ml_dtypes.bfloat16)
    # mask (cols 0:8) and q-bias (cols 8:14) fused into one [128,14] constant,
    # flattened t-major to match the kernel's (t p) -> p t view
    misc = np.concatenate([m.reshape(B, NKT, P).transpose(0, 2, 1),
                           np.broadcast_to(bq.reshape(KC, P).T[None],
                                           (B, P, KC))], axis=2)
    misc_flat = np.ascontiguousarray(misc.transpose(0, 2, 1)).reshape(B, -1)
    return [
        {"xt": xT[b], "msk": misc_flat[b], "wqkv": wqkv,
         "wout": wout, "beff": beff}
        for b in range(B)
    ]


def kernel(x, mask, W_qkv, b_qkv, W_out, b_out):
    nc = _build()
    maps = _in_maps(x, mask, W_qkv, b_qkv, W_out, b_out)
    res = run_bass_kernel_spmd(nc, maps, list(range(B))).results
    out = np.stack([np.asarray(res[b]["out"]).astype(np.float32)
                    for b in range(B)])
    return out


# revision 29
# speedup vs baseline: 1.1958x; 1.0197x over previous
"""BERT multi-head attention on 8 Trainium2 NeuronCores, data-parallel over batch.

Problem: x[8,1024,768] fp32, 12 heads, qkv + masked softmax attention + out proj.
Each core handles one batch element end-to-end; host gathers the 8 outputs.

Per-core layout strategy (S=1024, D=768, H=12, Dh=64):
  - all matmuls are bf16 x bf16 with f32 PSUM accumulation (same PE rate as
    f32r, half the DMA/SBUF): x and the weights are host-cast to bf16.
  - x is fed TRANSPOSED (xT [D,S]); q,k are produced transposed (qT/kT [D,S]);
    scores are computed transposed (scoresT [k,q], f32 psum, 2 heads packed
    per 128-row PE pass via tile_position).
  - ctx runs in [q-partition, Dh-free] orientation: the exp'd scores p [k,q]
    (bf16) are the matmul STATIONARY and v (bf16, mask folded in, plus a
    masked ones-column) is the moving operand, so every ctx matmul is a
    fully-utilized 128x128x65 tile and the softmax denominator rides along
    as output column 64. A PSUM bank holds one live accumulation group
    (start=True marks the whole 2KB zero-region), so each ctx tile's 8-kt
    accumulation runs as one back-to-back group; the 8 groups of q-half X
    are slotted into q-half X+1's score/exp stream (p tiles persist, 16 bufs).
  - softmax normalization is free: a DVE reciprocal of the denominator column
    plus per-partition tensor_scalar_mul during the PSUM->SBUF evacuation.
  - ctx[q, headA|headB] tiles are flipped to the [d-chunk, s] layout the
    output projection needs by ONE DMA xbar block-transpose per q-half
    ([128,512] -> 4x[128,128] blocks, 3D out AP; zero PE cost).
  - k-bias is dropped entirely: softmax over k is invariant to the per-q
    constant (q+bq)@bk, so only the q-bias is applied.
  - PE fill discipline: qk projection chunks and the output projection are
    chopped into ~512-cycle closures and pumped between attention matmuls so
    the PE never waits on the Activation engine's exp (1038ns/tile vs 643ns
    of attention matmuls per kt). The sweep is qh0-front so all six ctxT
    qh0 halves finish early and the s-tile 0..3 output projection overlaps
    the last three score sweeps.
  - PSUM: scores double-buffered (4 banks) + ctx accumulator (2 banks,
    [128, 8, 128] f32 so each 65-col group stays inside a bank) + a 2x1-bank
    fill pool shared by qk/v/out-proj = exactly 8 banks.
"""

import sys

import numpy as np

try:
    import concourse.bass as bass
except ImportError:  # pragma: no cover
    sys.path.insert(0, "/opt/trn_rl_repo")
    import concourse.bass as bass

from collections import deque
from contextlib import ExitStack

import concourse.tile as tile
from concourse import bacc, mybir
from concourse._compat import with_exitstack
from concourse.bass_utils import run_bass_kernel_spmd

F32 = mybir.dt.float32
BF16 = mybir.dt.bfloat16
EXP = mybir.ActivationFunctionType.Exp
COPY = mybir.ActivationFunctionType.Copy

B, S, D, H, DH, P = 8, 1024, 768, 12, 64, 128
KC = D // P          # 6 contraction chunks of 128
NKT = S // P         # 8 k-tiles of 128
NP = H // 2          # 6 head pairs
SCALE = 1.0 / np.sqrt(DH)


@with_exitstack
def _emit(ctx: ExitStack, tc, out, xT, wqkv, wout, beff, msk):
    nc = tc.nc

    const = ctx.enter_context(tc.tile_pool(name="const", bufs=1))
    persist = ctx.enter_context(tc.tile_pool(name="persist", bufs=1))
    p_pool = ctx.enter_context(tc.tile_pool(name="p", bufs=16))
    small = ctx.enter_context(tc.tile_pool(name="small", bufs=2))
    ctxn_pool = ctx.enter_context(tc.tile_pool(name="ctxn", bufs=2))
    osb_pool = ctx.enter_context(tc.tile_pool(name="osb", bufs=4))

    # ------------- inputs / constants -------------
    # DMA emission order == service order; the first score matmuls need
    # wq chunk 0, wq chunk 6 and x half 0, so those three go first.
    # wqkv arrives host-permuted as [q0|k0|q1..q5|k1..k5|v]: the first DMA
    # grabs both chunk-0 stationaries in one contiguous 256-col slice.
    wq_view = wqkv.rearrange("(c p) n -> p c n", p=P)   # [128, 6, 2304] bf16
    xT_view = xT.rearrange("(c p) s -> p c s", p=P)     # [128, 6, 1024] bf16
    wqk_sb = persist.tile([P, KC, 2 * D], BF16)         # permuted q|k layout
    xT_sb = persist.tile([P, KC, S], BF16)
    misc_sb = const.tile([P, NKT + KC], F32)            # mask cols 0:8, bq 8:14
    for h in range(3):  # chunk-pair pieces so the first matmuls can chase
        cs = slice(2 * h, 2 * h + 2)
        nc.sync.dma_start(wqk_sb[:, cs, 0:2 * P], wq_view[:, cs, 0:2 * P])
        nc.sync.dma_start(xT_sb[:, cs, 0:512], xT_view[:, cs, 0:512])
        if h == 0:
            nc.sync.dma_start(misc_sb[:], msk.rearrange("(t p) -> p t", p=P))
    wv_sb = persist.tile([P, KC, D], BF16)
    nc.sync.dma_start(wqk_sb[:, :, 7 * P:8 * P], wq_view[:, :, 7 * P:8 * P])
    nc.sync.dma_start(xT_sb[:, :, 512:1024], xT_view[:, :, 512:1024])
    nc.sync.dma_start(wv_sb[:, :, 0:384], wq_view[:, :, 2 * D:2 * D + 384])
    nc.sync.dma_start(wv_sb[:, :, 384:768], wq_view[:, :, 2 * D + 384:3 * D])
    nc.sync.dma_start(wqk_sb[:, :, 2 * P:7 * P], wq_view[:, :, 2 * P:7 * P])
    nc.sync.dma_start(wqk_sb[:, :, 8 * P:12 * P], wq_view[:, :, 8 * P:12 * P])
    wo_sb = persist.tile([P, KC, D], BF16)
    nc.sync.dma_start(wo_sb[:], wout.rearrange("(c p) n -> p c n", p=P))
    beff_bc = const.tile([P, D], F32)
    nc.sync.dma_start(beff_bc[:], beff.partition_broadcast(P))
    ones_sb = const.tile([P, 6], F32)
    nc.vector.memset(ones_sb[:], 1.0)

    _wq_off = {m: (0 if m == 0 else P if m == KC else
                   (1 + m) * P if m < KC else (m - KC + 6) * P)
               for m in range(2 * KC)}

    def wq_t(m, c):  # stationary slice for qk chunk m in the permuted layout
        o = _wq_off[m]
        return wqk_sb[:, c, o:o + P]

    qkT_sb = persist.tile([P, 2 * KC, S], BF16)    # chunks 0..5 = qT, 6..11 = kT
    v_sb = persist.tile([P, NKT, H, DH + 1], BF16)  # masked v + masked ones col
    ctxT0 = persist.tile([P, KC, 512], BF16)   # [d-chunk, s 0:512]
    ctxT1 = persist.tile([P, KC, 512], BF16)   # [d-chunk, s 512:1024]
    ctxT3d = [ctxT0[:].rearrange("p c (j q) -> p c j q", q=P),
              ctxT1[:].rearrange("p c (j q) -> p c j q", q=P)]

    def ctxT(c, qt):  # [128, 128] stationary block for s-tile qt
        t = ctxT0 if qt < 4 else ctxT1
        return t[:, c, (qt % 4) * P:(qt % 4 + 1) * P]

    # ------------- psum pools -------------
    ps_s = ctx.enter_context(tc.tile_pool(name="ps_s", bufs=2, space="PSUM"))
    ps_ctx = ctx.enter_context(tc.tile_pool(name="ps_ctx", bufs=1, space="PSUM"))
    ps_fill = ctx.enter_context(tc.tile_pool(name="ps_fill", bufs=2, space="PSUM"))

    # ------------- fill work: qk projection halves, out projection ----------
    # Each closure is ~512 PE cycles; pumped between attention matmuls so the
    # PE always has slack-independent work while ACT exps catch up.
    fill_q = deque()

    def pump(n):
        for _ in range(n):
            if not fill_q:
                return
            fill_q.popleft()()

    def qk_half_closures(m, n):
        # 6 accumulation matmuls into a 1-bank fill tile + a DVE evac
        st = {}

        def chunk(c):
            def go():
                if c == 0:
                    st["t"] = ps_fill.tile([P, 512], F32, tag="fill",
                                           name="qkfill")
                nc.tensor.matmul(
                    st["t"][:, 0:512],
                    wq_t(m, c),
                    xT_sb[:, c, n * 512:(n + 1) * 512],
                    start=(c == 0), stop=(c == KC - 1))
                if c == KC - 1:
                    dst = qkT_sb[:, m, n * 512:(n + 1) * 512]
                    if m < KC:  # q gets its bias; k-bias is softmax-invariant
                        nc.vector.tensor_scalar_add(dst, st["t"][:, 0:512],
                                                    misc_sb[:, NKT + m:NKT + m + 1])
                    else:
                        nc.vector.tensor_copy(dst, st["t"][:, 0:512])
            return go
        return [chunk(c) for c in range(KC)]

    # ----- V projection, one s-chunk, one half (6 heads), masked + ones col --
    def emit_v_st(st_i, half):
        pv = ps_fill.tile([P, 512], F32, tag="fill")
        for c in range(KC):
            nc.tensor.matmul(
                pv[:, 0:384],
                xT_sb[:, c, st_i * P:(st_i + 1) * P],
                wv_sb[:, c, half * 384:(half + 1) * 384],
                start=(c == 0), stop=(c == KC - 1))
        hs = slice(half * 6, (half + 1) * 6)
        nc.vector.tensor_scalar_mul(
            v_sb[:, st_i, hs, 0:DH],
            pv[:, 0:384].rearrange("p (h d) -> p h d", h=6),
            misc_sb[:, st_i:st_i + 1])
        nc.gpsimd.tensor_scalar_mul(
            v_sb[:, st_i, hs, DH:DH + 1],
            ones_sb[:].unsqueeze(2),
            misc_sb[:, st_i:st_i + 1])

    # ------------- out projection for one 128-row s-tile -------------
    def outproj_closures(qt):
        st = {}

        def mk(piece, c):
            def go():
                if c == 0:
                    st[piece] = ps_fill.tile([P, 512], F32, tag="fill",
                                             name="ofill")
                    if piece == 0:
                        st["o"] = osb_pool.tile([P, D], BF16, tag="o",
                                                name="osb")
                nc.tensor.matmul(
                    st[piece][:, 0:384],
                    ctxT(c, qt),
                    wo_sb[:, c, piece * 384:(piece + 1) * 384],
                    start=(c == 0), stop=(c >= KC - 2),
                    skip_group_check=(c == KC - 1))
                lo = piece * 384
                if c == KC - 2:
                    # fold the bias into the open psum group so the final
                    # evacuation is a cheap 2x-mode copy
                    nc.vector.tensor_add(st[piece][:, 0:384],
                                         st[piece][:, 0:384],
                                         beff_bc[:, lo:lo + 384])
                if c == KC - 1:
                    nc.vector.tensor_copy(st["o"][:, lo:lo + 384],
                                          st[piece][:, 0:384])
                    if piece == 1:
                        nc.sync.dma_start(out[qt * P:(qt + 1) * P, :],
                                          st["o"][:])
            return go
        order = ([(0, c) for c in range(KC - 1)]
                 + [(1, c) for c in range(KC - 1)]
                 + [(0, KC - 1), (1, KC - 1)])
        return [mk(piece, c) for piece, c in order]

    # ------------- attention, pipelined one q-half deep -------------
    def ctx_group(pair, ctx_t, p_list, slot):
        # slot 0..7 -> groups A0,B0,A1,B1,... (banks alternate, so each bank
        # has at most one live accumulation group)
        j, half = slot // 2, slot % 2
        h = 2 * pair + half
        jj = 4 * half + j
        for kt in range(NKT):
            nc.tensor.matmul(
                ctx_t[:, jj, 0:DH + 1],
                p_list[kt][:, half * 512 + j * P:half * 512 + (j + 1) * P],
                v_sb[:, kt, h, :],
                start=(kt == 0), stop=(kt == NKT - 1),
                skip_group_check=True)

    def finish_qh(pair, qh, ctx_t, fast=False):
        # normalization + evacuation + transpose: one strided DVE reciprocal,
        # 8 evac-muls, two xbar half-transposes. fast=True splits the muls
        # across DVE and Pool for the tail-critical last q-half.
        r_sb = small.tile([P, NKT], F32, tag="r")
        nc.vector.reciprocal(r_sb[:].unsqueeze(2), ctx_t[:, :, DH:DH + 1])
        cn = ctxn_pool.tile([P, 512], BF16, tag="cn", name="cn")

        def mul(j, eng):
            if eng is nc.scalar:
                nc.scalar.activation(cn[:, j * P:j * P + DH],
                                     ctx_t[:, j, 0:DH], COPY,
                                     scale=r_sb[:, j:j + 1])
                nc.scalar.activation(cn[:, j * P + DH:(j + 1) * P],
                                     ctx_t[:, 4 + j, 0:DH], COPY,
                                     scale=r_sb[:, 4 + j:4 + j + 1])
                return
            eng.tensor_scalar_mul(cn[:, j * P:j * P + DH],
                                  ctx_t[:, j, 0:DH], r_sb[:, j:j + 1])
            eng.tensor_scalar_mul(cn[:, j * P + DH:(j + 1) * P],
                                  ctx_t[:, 4 + j, 0:DH],
                                  r_sb[:, 4 + j:4 + j + 1])

        e1 = nc.scalar if fast else nc.vector
        mul(0, nc.vector)
        mul(1, e1)
        nc.sync.dma_start(ctxT3d[qh][:, pair, 0:2, :],
                          cn[:, 0:256], transpose=True)
        mul(2, nc.vector)
        mul(3, e1)
        nc.sync.dma_start(ctxT3d[qh][:, pair, 2:4, :],
                          cn[:, 256:512], transpose=True)

    def attention(pair, qh, prev, v_half=None, ppk=2, pre_kt=None):
        # returns (pair, qh, p_list) to be consumed by the next call
        if isinstance(ppk, int):
            ppk = [ppk] * NKT
        pre_kt = pre_kt or {}
        qs = slice(qh * 512, (qh + 1) * 512)
        ctx_t = None
        p_list = []
        for kt in range(NKT):
            for fn in pre_kt.get(kt, ()):
                fn()
            s_ps = ps_s.tile([P, 1024], F32, tag="s_ps")
            nc.tensor.matmul(
                s_ps[:, 0:512],
                qkT_sb[0:DH, KC + pair, kt * P:(kt + 1) * P],
                qkT_sb[0:DH, pair, qs],
                start=True, stop=True, tile_position=(0, 0))
            nc.tensor.matmul(
                s_ps[:, 512:1024],
                qkT_sb[DH:P, KC + pair, kt * P:(kt + 1) * P],
                qkT_sb[DH:P, pair, qs],
                start=True, stop=True, tile_position=(DH, 0))
            p_t = p_pool.tile([P, 1024], BF16)
            nc.scalar.activation(p_t[:], s_ps[:], EXP, bias=0.0, scale=SCALE)
            p_list.append(p_t)
            if v_half is not None and kt > 3:
                emit_v_st(kt - 4, v_half)
            if prev is not None:
                # ctx groups sit at kts 2..5 (two per kt, banks alternating):
                # late enough that the previous item's last exp has landed,
                # early enough that the finish chain at kt6 has 4 kts of
                # slack before the next item's first ctx write
                if ctx_t is None:
                    ctx_t = ps_ctx.tile([P, NKT, P], F32, tag="ctx")
                for slot in ((), (), (0, 1), (2, 3), (4, 5), (6, 7), (), ())[kt]:
                    ctx_group(prev[0], ctx_t, prev[2], slot)
                if kt == 6:
                    finish_qh(prev[0], prev[1], ctx_t)
            pump(ppk[kt])
        return (pair, qh, p_list)

    def drain_last(prev):
        ctx_t = ps_ctx.tile([P, NKT, P], F32, tag="ctx")
        for slot in range(NKT):
            pump(2)
            ctx_group(prev[0], ctx_t, prev[2], slot)
        finish_qh(prev[0], prev[1], ctx_t, fast=True)

    # ------------- phase structure -------------
    with nc.allow_low_precision(reason="bf16 stores are within the 2e-2 gate"):
        # pre-attention: q(0, half0) + k(6, half0), interleaved per chunk so
        # the matmuls chase the x DMA pieces
        tq = ps_fill.tile([P, 512], F32, tag="fill")
        tk = ps_fill.tile([P, 512], F32, tag="fill")
        for c in range(KC):
            nc.tensor.matmul(tq[:, 0:512], wq_t(0, c), xT_sb[:, c, 0:512],
                             start=(c == 0), stop=(c == KC - 1))
            nc.tensor.matmul(tk[:, 0:512], wq_t(KC, c), xT_sb[:, c, 0:512],
                             start=(c == 0), stop=(c == KC - 1))
        nc.vector.tensor_scalar_add(qkT_sb[:, 0, 0:512], tq[:, 0:512],
                                    misc_sb[:, NKT:NKT + 1])
        nc.vector.tensor_copy(qkT_sb[:, KC, 0:512], tk[:, 0:512])

        # sweep: qh0-front so every ctxT qh0 half is transposed by item 9 and
        # the s-tile 0..3 out projection pumps through items 10-12.
        # fill deadlines (emission order, in-order PE): a half must be fully
        # emitted before the first score matmul that reads it.
        k61 = qk_half_closures(KC, 1)   # k(6) half1: inline before kt4, needs
        #                                 x half1 which lands mid-item-1
        P18 = [3, 3, 1, 1, 1, 1, 4, 4]
        P12 = [2, 2, 1, 1, 1, 1, 2, 2]
        P16 = [3, 3, 1, 1, 1, 1, 3, 3]
        sweep = [
            # (pair, qh, v_half, pumped_halves, ppk)
            (0, 0, 0, [(KC + 1, 0)], [2, 2, 1, 1, 0, 0, 0, 0]),
            (0, 1, None, [(KC + 1, 1), (1, 0), (KC + 2, 0)], P18),
            (1, 0, 1, [], 0),
            (1, 1, None, [(KC + 2, 1), (2, 0), (KC + 3, 0)], P18),
            (2, 0, None, [(KC + 3, 1), (3, 0), (KC + 4, 0)], P18),
            (3, 0, None, [(KC + 4, 1), (4, 0), (KC + 5, 0)], P18),
            (4, 0, None, [(KC + 5, 1), (5, 0)], P12),
            (5, 0, None, [(2, 1), (3, 1)], P12),
            (2, 1, None, [(4, 1), (5, 1)], P12),
            (3, 1, "op", [], P16),
            (4, 1, None, [], P16),
            (5, 1, None, [], P16),
        ]
        posts = {(0, 0): [(0, 1)], (1, 0): [(1, 1)]}
        prev = None
        for i, (pair, qh, vh, pumped, ppk) in enumerate(sweep):
            if vh == "op":
                vh = None
                for qt in range(4):
                    fill_q.extend(outproj_closures(qt))
            for key in pumped:
                fill_q.extend(qk_half_closures(*key))
            pre = {}
            if i == 0:
                # k(6,1) emitted compactly before kt4 (its x-half-1 input
                # lands mid-item); v s-chunks run four kts late (the wv and
                # x DMAs land mid-item), spilling into the next item's pre
                pre = {4: k61}
            if i in (1, 3):
                vh_prev = 0 if i == 1 else 1
                pre = {0: [lambda h=vh_prev: emit_v_st(4, h),
                           lambda h=vh_prev: emit_v_st(5, h)],
                       1: [lambda h=vh_prev: emit_v_st(6, h),
                           lambda h=vh_prev: emit_v_st(7, h)]}
            prev = attention(pair, qh, prev, v_half=vh, ppk=ppk, pre_kt=pre)
            for key in posts.get((pair, qh), ()):
                for fn in qk_half_closures(*key):
                    fn()
        drain_last(prev)
        pump(len(fill_q))
        pieces = []   # (qt, piece, psum ap)
        for qt in (4, 5):
            t = ps_s.tile([P, 1024], F32, tag="s_ps", name="ost")
            pieces += [(qt, 0, t[:, 0:384]), (qt, 1, t[:, 512:896])]
        tc6 = ps_ctx.tile([P, NKT, P], F32, tag="ctx", name="oct")
        t6 = tc6[:].rearrange("p j q -> p (j q)")
        pieces += [(6, 0, t6[:, 0:384]), (6, 1, t6[:, 512:896])]
        for qt in (7,):
            ta = ps_fill.tile([P, 512], F32, tag="fill", name="ofa")
            tb = ps_fill.tile([P, 512], F32, tag="fill", name="ofb")
            pieces += [(qt, 0, ta[:, 0:384]), (qt, 1, tb[:, 0:384])]
        for qt, piece, ap in pieces:        # pairs 0..4 stream immediately
            for c in range(KC - 1):
                nc.tensor.matmul(ap, ctxT(c, qt),
                                 wo_sb[:, c, piece * 384:(piece + 1) * 384],
                                 start=(c == 0), stop=(c == KC - 2))
            nc.vector.tensor_add(ap, ap, beff_bc[:, piece * 384:
                                                 piece * 384 + 384])
        osbs = {}
        for qt, piece, ap in pieces:        # pair-5 chunk gated on transpose
            nc.tensor.matmul(ap, ctxT(KC - 1, qt),
                             wo_sb[:, KC - 1, piece * 384:(piece + 1) * 384],
                             start=False, stop=True, skip_group_check=True)
            if piece == 0:
                osbs[qt] = osb_pool.tile([P, D], BF16, tag="o", name="osb")
            lo = piece * 384
            if piece == 0:
                nc.vector.tensor_copy(osbs[qt][:, lo:lo + 384], ap)
            else:
                nc.scalar.activation(osbs[qt][:, lo:lo + 384], ap, COPY)
            if piece == 1:
                nc.sync.dma_start(out[qt * P:(qt + 1) * P, :], osbs[qt][:])


_CACHE = {}


def _build():
    if "nc" in _CACHE:
        return _CACHE["nc"]
    nc = bacc.Bacc("TRN2", target_bir_lowering=False, debug=False,
                   num_devices=B)
    xT = nc.dram_tensor("xt", [D, S], BF16, kind="ExternalInput").ap()
    wqkv = nc.dram_tensor("wqkv", [D, 3 * D], BF16, kind="ExternalInput").ap()
    wout = nc.dram_tensor("wout", [D, D], BF16, kind="ExternalInput").ap()
    beff = nc.dram_tensor("beff", [D], F32, kind="ExternalInput").ap()
    msk = nc.dram_tensor("msk", [P * (NKT + KC)], F32,
                         kind="ExternalInput").ap()
    out = nc.dram_tensor("out", [S, D], BF16, kind="ExternalOutput").ap()
    with tile.TileContext(nc) as tc:
        _emit(tc, out, xT, wqkv, wout, beff, msk)
    nc.compile()
    _CACHE["nc"] = nc
    return nc


def _in_maps(x, mask, W_qkv, b_qkv, W_out, b_out):
    import ml_dtypes
    xT = np.ascontiguousarray(np.transpose(
        np.asarray(x, dtype=np.float32), (0, 2, 1))).astype(ml_dtypes.bfloat16)
    m = np.asarray(mask).reshape(B, S).astype(np.float32)
    bq = np.asarray(b_qkv, np.float32)[:D]
    beff = (np.asarray(b_qkv, np.float64)[2 * D:] @ np.asarray(W_out, np.float64)
            + np.asarray(b_out, np.float64)).astype(np.float32)
    w = np.asarray(W_qkv, np.float32)
    # column permutation [q0|k0|q1..q5|k1..k5|v] so the kernel's first weight
    # DMA is one contiguous slice covering both chunk-0 stationaries
    wqkv = np.concatenate(
        [w[:, 0:P], w[:, D:D + P], w[:, P:D], w[:, D + P:2 * D],
         w[:, 2 * D:]], axis=1).astype(ml_dtypes.bfloat16)
    wout = np.asarray(W_out, np.float32).astype(ml_dtypes.bfloat16)
    # mask (cols 0:8) and q-bias (cols 8:14) fused into one [128,14] constant,
    # flattened t-major to match the kernel's (t p) -> p t view
    misc = np.concatenate([m.reshape(B, NKT, P).transpose(0, 2, 1),
                           np.broadcast_to(bq.reshape(KC, P).T[None],
                                           (B, P, KC))], axis=2)
    misc_flat = np.ascontiguousarray(misc.transpose(0, 2, 1)).reshape(B, -1)
    return [
        {"xt": xT[b], "msk": misc_flat[b], "wqkv": wqkv,
         "wout": wout, "beff": beff}
        for b in range(B)
    ]


def kernel(x, mask, W_qkv, b_qkv, W_out, b_out):
    nc = _build()
    maps = _in_maps(x, mask, W_qkv, b_qkv, W_out, b_out)
    res = run_bass_kernel_spmd(nc, maps, list(range(B))).results
    out = np.stack([np.asarray(res[b]["out"]).astype(np.float32)
                    for b in range(B)])
    return out


# revision 38
# speedup vs baseline: 1.2000x; 1.0035x over previous
"""BERT multi-head attention on 8 Trainium2 NeuronCores, data-parallel over batch.

Problem: x[8,1024,768] fp32, 12 heads, qkv + masked softmax attention + out proj.
Each core handles one batch element end-to-end; host gathers the 8 outputs.

Per-core layout strategy (S=1024, D=768, H=12, Dh=64):
  - all matmuls are bf16 x bf16 with f32 PSUM accumulation (same PE rate as
    f32r, half the DMA/SBUF): x and the weights are host-cast to bf16.
  - x is fed TRANSPOSED (xT [D,S]); q,k are produced transposed (qT/kT [D,S]);
    scores are computed transposed (scoresT [k,q], f32 psum, 2 heads packed
    per 128-row PE pass via tile_position).
  - ctx runs in [q-partition, Dh-free] orientation: the exp'd scores p [k,q]
    (bf16) are the matmul STATIONARY and v (bf16, mask folded in, plus a
    masked ones-column) is the moving operand, so every ctx matmul is a
    fully-utilized 128x128x65 tile and the softmax denominator rides along
    as output column 64. A PSUM bank holds one live accumulation group
    (start=True marks the whole 2KB zero-region), so each ctx tile's 8-kt
    accumulation runs as one back-to-back group; the 8 groups of q-half X
    are slotted into q-half X+1's score/exp stream (p tiles persist, 16 bufs).
  - softmax normalization is free: a DVE reciprocal of the denominator column
    plus per-partition tensor_scalar_mul during the PSUM->SBUF evacuation.
  - ctx[q, headA|headB] tiles are flipped to the [d-chunk, s] layout the
    output projection needs by ONE DMA xbar block-transpose per q-half
    ([128,512] -> 4x[128,128] blocks, 3D out AP; zero PE cost).
  - k-bias is dropped entirely: softmax over k is invariant to the per-q
    constant (q+bq)@bk, so only the q-bias is applied.
  - PE fill discipline: qk projection chunks and the output projection are
    chopped into ~512-cycle closures and pumped between attention matmuls so
    the PE never waits on the Activation engine's exp (1038ns/tile vs 643ns
    of attention matmuls per kt). The sweep is qh0-front so all six ctxT
    qh0 halves finish early and the s-tile 0..3 output projection overlaps
    the last three score sweeps.
  - PSUM: scores double-buffered (4 banks) + ctx accumulator (2 banks,
    [128, 8, 128] f32 so each 65-col group stays inside a bank) + a 2x1-bank
    fill pool shared by qk/v/out-proj = exactly 8 banks.
"""

import sys

import numpy as np

try:
    import concourse.bass as bass
except ImportError:  # pragma: no cover
    sys.path.insert(0, "/opt/trn_rl_repo")
    import concourse.bass as bass

from collections import deque
from contextlib import ExitStack

import concourse.tile as tile
from concourse import bacc, mybir
from concourse._compat import with_exitstack
from concourse.bass_utils import run_bass_kernel_spmd

F32 = mybir.dt.float32
BF16 = mybir.dt.bfloat16
EXP = mybir.ActivationFunctionType.Exp
COPY = mybir.ActivationFunctionType.Copy

B, S, D, H, DH, P = 8, 1024, 768, 12, 64, 128
KC = D // P          # 6 contraction chunks of 128
NKT = S // P         # 8 k-tiles of 128
NP = H // 2          # 6 head pairs
SCALE = 1.0 / np.sqrt(DH)


@with_exitstack
def _emit(ctx: ExitStack, tc, out, xT, wqkv, wout, beff, msk):
    nc = tc.nc

    const = ctx.enter_context(tc.tile_pool(name="const", bufs=1))
    persist = ctx.enter_context(tc.tile_pool(name="persist", bufs=1))
    p_pool = ctx.enter_context(tc.tile_pool(name="p", bufs=16))
    small = ctx.enter_context(tc.tile_pool(name="small", bufs=2))
    ctxn_pool = ctx.enter_context(tc.tile_pool(name="ctxn", bufs=2))
    osb_pool = ctx.enter_context(tc.tile_pool(name="osb", bufs=4))

    # ------------- inputs / constants -------------
    # DMA emission order == service order; the first score matmuls need
    # wq chunk 0, wq chunk 6 and x half 0, so those three go first.
    # wqkv arrives host-permuted as [q0|k0|q1..q5|k1..k5|v]: the first DMA
    # grabs both chunk-0 stationaries in one contiguous 256-col slice.
    wq_view = wqkv.rearrange("(c p) n -> p c n", p=P)   # [128, 6, 2304] bf16
    xT_view = xT.rearrange("(c p) s -> p c s", p=P)     # [128, 6, 1024] bf16
    wqk_sb = persist.tile([P, KC, 2 * D], BF16)         # permuted q|k layout
    xT_sb = persist.tile([P, KC, S], BF16)
    misc_sb = const.tile([P, NKT + KC], F32)            # mask cols 0:8, bq 8:14
    for h in range(3):  # chunk-pair pieces so the first matmuls can chase
        cs = slice(2 * h, 2 * h + 2)
        nc.sync.dma_start(wqk_sb[:, cs, 0:2 * P], wq_view[:, cs, 0:2 * P])
        nc.sync.dma_start(xT_sb[:, cs, 0:512], xT_view[:, cs, 0:512])
        if h == 0:
            nc.sync.dma_start(misc_sb[:], msk.rearrange("(t p) -> p t", p=P))
    wv_sb = persist.tile([P, KC, D], BF16)
    nc.sync.dma_start(wqk_sb[:, :, 7 * P:8 * P], wq_view[:, :, 7 * P:8 * P])
    nc.sync.dma_start(xT_sb[:, :, 512:1024], xT_view[:, :, 512:1024])
    nc.sync.dma_start(wv_sb[:, :, 0:384], wq_view[:, :, 2 * D:2 * D + 384])
    nc.sync.dma_start(wv_sb[:, :, 384:768], wq_view[:, :, 2 * D + 384:3 * D])
    nc.sync.dma_start(wqk_sb[:, :, 2 * P:7 * P], wq_view[:, :, 2 * P:7 * P])
    nc.sync.dma_start(wqk_sb[:, :, 8 * P:12 * P], wq_view[:, :, 8 * P:12 * P])
    wo_sb = persist.tile([P, KC, D], BF16)
    nc.sync.dma_start(wo_sb[:], wout.rearrange("(c p) n -> p c n", p=P))
    beff_bc = const.tile([P, D], F32)
    nc.sync.dma_start(beff_bc[:], beff.partition_broadcast(P))
    ones_sb = const.tile([P, 6], F32)
    nc.vector.memset(ones_sb[:], 1.0)

    _wq_off = {m: (0 if m == 0 else P if m == KC else
                   (1 + m) * P if m < KC else (m - KC + 6) * P)
               for m in range(2 * KC)}

    def wq_t(m, c):  # stationary slice for qk chunk m in the permuted layout
        o = _wq_off[m]
        return wqk_sb[:, c, o:o + P]

    qkT_sb = persist.tile([P, 2 * KC, S], BF16)    # chunks 0..5 = qT, 6..11 = kT
    v_sb = persist.tile([P, NKT, H, DH + 1], BF16)  # masked v + masked ones col
    ctxT0 = persist.tile([P, KC, 512], BF16)   # [d-chunk, s 0:512]
    ctxT1 = persist.tile([P, KC, 512], BF16)   # [d-chunk, s 512:1024]
    ctxT3d = [ctxT0[:].rearrange("p c (j q) -> p c j q", q=P),
              ctxT1[:].rearrange("p c (j q) -> p c j q", q=P)]

    def ctxT(c, qt):  # [128, 128] stationary block for s-tile qt
        t = ctxT0 if qt < 4 else ctxT1
        return t[:, c, (qt % 4) * P:(qt % 4 + 1) * P]

    # ------------- psum pools -------------
    ps_s = ctx.enter_context(tc.tile_pool(name="ps_s", bufs=2, space="PSUM"))
    ps_ctx = ctx.enter_context(tc.tile_pool(name="ps_ctx", bufs=1, space="PSUM"))
    ps_fill = ctx.enter_context(tc.tile_pool(name="ps_fill", bufs=2, space="PSUM"))

    # ------------- fill work: qk projection halves, out projection ----------
    # Each closure is ~512 PE cycles; pumped between attention matmuls so the
    # PE always has slack-independent work while ACT exps catch up.
    fill_q = deque()

    def pump(n):
        for _ in range(n):
            if not fill_q:
                return
            fill_q.popleft()()

    def qk_half_closures(m, n):
        # 6 accumulation matmuls into a 1-bank fill tile + a DVE evac
        st = {}

        def chunk(c):
            def go():
                if c == 0:
                    st["t"] = ps_fill.tile([P, 512], F32, tag="fill",
                                           name="qkfill")
                nc.tensor.matmul(
                    st["t"][:, 0:512],
                    wq_t(m, c),
                    xT_sb[:, c, n * 512:(n + 1) * 512],
                    start=(c == 0), stop=(c == KC - 1))
                if c == KC - 1:
                    dst = qkT_sb[:, m, n * 512:(n + 1) * 512]
                    if m < KC:  # q gets its bias; k-bias is softmax-invariant
                        nc.vector.tensor_scalar_add(dst, st["t"][:, 0:512],
                                                    misc_sb[:, NKT + m:NKT + m + 1])
                    else:
                        nc.vector.tensor_copy(dst, st["t"][:, 0:512])
            return go
        return [chunk(c) for c in range(KC)]

    # ----- V projection, one s-chunk, one half (6 heads), masked + ones col --
    def emit_v_st(st_i, half):
        pv = ps_fill.tile([P, 512], F32, tag="fill")
        for c in range(KC):
            nc.tensor.matmul(
                pv[:, 0:384],
                xT_sb[:, c, st_i * P:(st_i + 1) * P],
                wv_sb[:, c, half * 384:(half + 1) * 384],
                start=(c == 0), stop=(c == KC - 1))
        hs = slice(half * 6, (half + 1) * 6)
        nc.vector.tensor_scalar_mul(
            v_sb[:, st_i, hs, 0:DH],
            pv[:, 0:384].rearrange("p (h d) -> p h d", h=6),
            misc_sb[:, st_i:st_i + 1])
        nc.gpsimd.tensor_scalar_mul(
            v_sb[:, st_i, hs, DH:DH + 1],
            ones_sb[:].unsqueeze(2),
            misc_sb[:, st_i:st_i + 1])

    # ------------- out projection for one 128-row s-tile -------------
    def outproj_closures(qt):
        st = {}

        def mk(piece, c):
            def go():
                if c == 0:
                    st[piece] = ps_fill.tile([P, 512], F32, tag="fill",
                                             name="ofill")
                    if piece == 0:
                        st["o"] = osb_pool.tile([P, D], BF16, tag="o",
                                                name="osb")
                nc.tensor.matmul(
                    st[piece][:, 0:384],
                    ctxT(c, qt),
                    wo_sb[:, c, piece * 384:(piece + 1) * 384],
                    start=(c == 0), stop=(c == KC - 1))
                lo = piece * 384
                if c == KC - 1:
                    nc.vector.tensor_add(st["o"][:, lo:lo + 384],
                                         st[piece][:, 0:384],
                                         beff_bc[:, lo:lo + 384])
                    if piece == 1:
                        nc.sync.dma_start(out[qt * P:(qt + 1) * P, :],
                                          st["o"][:])
            return go
        order = ([(0, c) for c in range(KC - 1)]
                 + [(1, c) for c in range(KC - 1)]
                 + [(0, KC - 1), (1, KC - 1)])
        return [mk(piece, c) for piece, c in order]

    # ------------- attention, pipelined one q-half deep -------------
    def ctx_group(pair, ctx_t, p_list, slot):
        # slot 0..7 -> groups A0,B0,A1,B1,... (banks alternate, so each bank
        # has at most one live accumulation group)
        j, half = slot // 2, slot % 2
        h = 2 * pair + half
        jj = 4 * half + j
        for kt in range(NKT):
            nc.tensor.matmul(
                ctx_t[:, jj, 0:DH + 1],
                p_list[kt][:, half * 512 + j * P:half * 512 + (j + 1) * P],
                v_sb[:, kt, h, :],
                start=(kt == 0), stop=(kt == NKT - 1),
                skip_group_check=True)

    def finish_qh(pair, qh, ctx_t, fast=False):
        # normalization + evacuation + transpose: one strided DVE reciprocal,
        # 8 evac-muls, two xbar half-transposes. fast=True splits the muls
        # across DVE and Pool for the tail-critical last q-half.
        r_sb = small.tile([P, NKT], F32, tag="r")
        nc.vector.reciprocal(r_sb[:].unsqueeze(2), ctx_t[:, :, DH:DH + 1])
        cn = ctxn_pool.tile([P, 512], BF16, tag="cn", name="cn")

        def mul(j, eng):
            if eng is nc.scalar:
                nc.scalar.activation(cn[:, j * P:j * P + DH],
                                     ctx_t[:, j, 0:DH], COPY,
                                     scale=r_sb[:, j:j + 1])
                nc.scalar.activation(cn[:, j * P + DH:(j + 1) * P],
                                     ctx_t[:, 4 + j, 0:DH], COPY,
                                     scale=r_sb[:, 4 + j:4 + j + 1])
                return
            eng.tensor_scalar_mul(cn[:, j * P:j * P + DH],
                                  ctx_t[:, j, 0:DH], r_sb[:, j:j + 1])
            eng.tensor_scalar_mul(cn[:, j * P + DH:(j + 1) * P],
                                  ctx_t[:, 4 + j, 0:DH],
                                  r_sb[:, 4 + j:4 + j + 1])

        e1 = nc.scalar if fast else nc.vector
        mul(0, nc.vector)
        mul(1, e1)
        nc.sync.dma_start(ctxT3d[qh][:, pair, 0:2, :],
                          cn[:, 0:256], transpose=True)
        mul(2, nc.vector)
        mul(3, e1)
        nc.sync.dma_start(ctxT3d[qh][:, pair, 2:4, :],
                          cn[:, 256:512], transpose=True)

    def attention(pair, qh, prev, v_half=None, ppk=2, pre_kt=None):
        # returns (pair, qh, p_list) to be consumed by the next call
        if isinstance(ppk, int):
            ppk = [ppk] * NKT
        pre_kt = pre_kt or {}
        qs = slice(qh * 512, (qh + 1) * 512)
        ctx_t = None
        p_list = []
        for kt in range(NKT):
            for fn in pre_kt.get(kt, ()):
                fn()
            s_ps = ps_s.tile([P, 1024], F32, tag="s_ps")
            nc.tensor.matmul(
                s_ps[:, 0:512],
                qkT_sb[0:DH, KC + pair, kt * P:(kt + 1) * P],
                qkT_sb[0:DH, pair, qs],
                start=True, stop=True, tile_position=(0, 0))
            nc.tensor.matmul(
                s_ps[:, 512:1024],
                qkT_sb[DH:P, KC + pair, kt * P:(kt + 1) * P],
                qkT_sb[DH:P, pair, qs],
                start=True, stop=True, tile_position=(DH, 0))
            p_t = p_pool.tile([P, 1024], BF16)
            nc.scalar.activation(p_t[:], s_ps[:], EXP, bias=0.0, scale=SCALE)
            p_list.append(p_t)
            if v_half is not None and kt > 3:
                emit_v_st(kt - 4, v_half)
            if prev is not None:
                # ctx groups sit at kts 2..5 (two per kt, banks alternating):
                # late enough that the previous item's last exp has landed,
                # early enough that the finish chain at kt6 has 4 kts of
                # slack before the next item's first ctx write
                if ctx_t is None:
                    ctx_t = ps_ctx.tile([P, NKT, P], F32, tag="ctx")
                for slot in ((), (), (0, 1), (2, 3), (4, 5), (6, 7), (), ())[kt]:
                    ctx_group(prev[0], ctx_t, prev[2], slot)
                if kt == 6:
                    finish_qh(prev[0], prev[1], ctx_t)
            pump(ppk[kt])
        return (pair, qh, p_list)

    def drain_last(prev):
        ctx_t = ps_ctx.tile([P, NKT, P], F32, tag="ctx")
        for slot in range(NKT):
            pump(2)
            ctx_group(prev[0], ctx_t, prev[2], slot)
        finish_qh(prev[0], prev[1], ctx_t, fast=True)

    # ------------- phase structure -------------
    with nc.allow_low_precision(reason="bf16 stores are within the 2e-2 gate"):
        # pre-attention: q(0, half0) + k(6, half0), interleaved per chunk so
        # the matmuls chase the x DMA pieces
        tq = ps_fill.tile([P, 512], F32, tag="fill")
        tk = ps_fill.tile([P, 512], F32, tag="fill")
        for c in range(KC):
            nc.tensor.matmul(tq[:, 0:512], wq_t(0, c), xT_sb[:, c, 0:512],
                             start=(c == 0), stop=(c == KC - 1))
            nc.tensor.matmul(tk[:, 0:512], wq_t(KC, c), xT_sb[:, c, 0:512],
                             start=(c == 0), stop=(c == KC - 1))
        nc.vector.tensor_scalar_add(qkT_sb[:, 0, 0:512], tq[:, 0:512],
                                    misc_sb[:, NKT:NKT + 1])
        nc.vector.tensor_copy(qkT_sb[:, KC, 0:512], tk[:, 0:512])

        # sweep: qh0-front so every ctxT qh0 half is transposed by item 9 and
        # the s-tile 0..3 out projection pumps through items 10-12.
        # fill deadlines (emission order, in-order PE): a half must be fully
        # emitted before the first score matmul that reads it.
        k61 = qk_half_closures(KC, 1)   # k(6) half1: inline before kt4, needs
        #                                 x half1 which lands mid-item-1
        P18 = [3, 3, 1, 1, 1, 1, 4, 4]
        P12 = [2, 2, 1, 1, 1, 1, 2, 2]
        P16 = [3, 3, 1, 1, 1, 1, 3, 3]
        sweep = [
            # (pair, qh, v_half, pumped_halves, ppk)
            (0, 0, 0, [(KC + 1, 0)], [2, 2, 1, 1, 0, 0, 0, 0]),
            (0, 1, None, [(KC + 1, 1), (1, 0), (KC + 2, 0)], P18),
            (1, 0, 1, [], 0),
            (1, 1, None, [(KC + 2, 1), (2, 0), (KC + 3, 0)], P18),
            (2, 0, None, [(KC + 3, 1), (3, 0), (KC + 4, 0)], P18),
            (3, 0, None, [(KC + 4, 1), (4, 0), (KC + 5, 0)], P18),
            (4, 0, None, [(KC + 5, 1), (5, 0)], P12),
            (5, 0, None, [(2, 1), (3, 1)], P12),
            (2, 1, None, [(4, 1), (5, 1)], P12),
            (3, 1, "op", [], P16),
            (4, 1, None, [], P16),
            (5, 1, None, [], P16),
        ]
        posts = {(0, 0): [(0, 1)], (1, 0): [(1, 1)]}
        prev = None
        for i, (pair, qh, vh, pumped, ppk) in enumerate(sweep):
            if vh == "op":
                vh = None
                for qt in range(4):
                    fill_q.extend(outproj_closures(qt))
            for key in pumped:
                fill_q.extend(qk_half_closures(*key))
            pre = {}
            if i == 0:
                # k(6,1) emitted compactly before kt4 (its x-half-1 input
                # lands mid-item); v s-chunks run four kts late (the wv and
                # x DMAs land mid-item), spilling into the next item's pre
                pre = {4: k61}
            if i in (1, 3):
                vh_prev = 0 if i == 1 else 1
                pre = {0: [lambda h=vh_prev: emit_v_st(4, h),
                           lambda h=vh_prev: emit_v_st(5, h)],
                       1: [lambda h=vh_prev: emit_v_st(6, h),
                           lambda h=vh_prev: emit_v_st(7, h)]}
            prev = attention(pair, qh, prev, v_half=vh, ppk=ppk, pre_kt=pre)
            for key in posts.get((pair, qh), ()):
                for fn in qk_half_closures(*key):
                    fn()
        t4 = ps_s.tile([P, 1024], F32, tag="s_ps", name="ost")
        head = [(4, 0, t4[:, 0:384]), (4, 1, t4[:, 512:896])]
        for qt, piece, ap in head:
            for c in range(KC - 1):
                nc.tensor.matmul(ap, ctxT(c, qt),
                                 wo_sb[:, c, piece * 384:(piece + 1) * 384],
                                 start=(c == 0), stop=(c == KC - 2))
            nc.vector.tensor_add(ap, ap, beff_bc[:, piece * 384:
                                                 piece * 384 + 384])
        drain_last(prev)
        pump(len(fill_q))
        pieces = list(head)   # (qt, piece, psum ap)
        for qt in (5,):
            t = ps_s.tile([P, 1024], F32, tag="s_ps", name="ost")
            pieces += [(qt, 0, t[:, 0:384]), (qt, 1, t[:, 512:896])]
        tc6 = ps_ctx.tile([P, NKT, P], F32, tag="ctx", name="oct")
        t6 = tc6[:].rearrange("p j q -> p (j q)")
        pieces += [(6, 0, t6[:, 0:384]), (6, 1, t6[:, 512:896])]
        for qt in (7,):
            ta = ps_fill.tile([P, 512], F32, tag="fill", name="ofa")
            tb = ps_fill.tile([P, 512], F32, tag="fill", name="ofb")
            pieces += [(qt, 0, ta[:, 0:384]), (qt, 1, tb[:, 0:384])]
        for qt, piece, ap in pieces[2:]:    # pairs 0..4 stream immediately
            for c in range(KC - 1):
                nc.tensor.matmul(ap, ctxT(c, qt),
                                 wo_sb[:, c, piece * 384:(piece + 1) * 384],
                                 start=(c == 0), stop=(c == KC - 2))
            nc.vector.tensor_add(ap, ap, beff_bc[:, piece * 384:
                                                 piece * 384 + 384])
        osbs = {}
        for qt, piece, ap in pieces:        # pair-5 chunk gated on transpose
            nc.tensor.matmul(ap, ctxT(KC - 1, qt),
                             wo_sb[:, KC - 1, piece * 384:(piece + 1) * 384],
                             start=False, stop=True, skip_group_check=True)
            if piece == 0:
                osbs[qt] = osb_pool.tile([P, D], BF16, tag="o", name="osb")
            lo = piece * 384
            if piece == 0:
                nc.vector.tensor_copy(osbs[qt][:, lo:lo + 384], ap)
            else:
                nc.scalar.activation(osbs[qt][:, lo:lo + 384], ap, COPY)
            if piece == 1:
                nc.sync.dma_start(out[qt * P:(qt + 1) * P, :], osbs[qt][:])


_CACHE = {}


def _build():
    if "nc" in _CACHE:
        return _CACHE["nc"]
    nc = bacc.Bacc("TRN2", target_bir_lowering=False, debug=False,
                   num_devices=B)
    xT = nc.dram_tensor("xt", [D, S], BF16, kind="ExternalInput").ap()
    wqkv = nc.dram_tensor("wqkv", [D, 3 * D], BF16, kind="ExternalInput").ap()
    wout = nc.dram_tensor("wout", [D, D], BF16, kind="ExternalInput").ap()
    beff = nc.dram_tensor("beff", [D], F32, kind="ExternalInput").ap()
    msk = nc.dram_tensor("msk", [P * (NKT + KC)], F32,
                         kind="ExternalInput").ap()
    out = nc.dram_tensor("out", [S, D], BF16, kind="ExternalOutput").ap()
    with tile.TileContext(nc) as tc:
        _emit(tc, out, xT, wqkv, wout, beff, msk)
    nc.compile()
    _CACHE["nc"] = nc
    return nc


def _in_maps(x, mask, W_qkv, b_qkv, W_out, b_out):
    import ml_dtypes
    xT = np.ascontiguousarray(np.transpose(
        np.asarray(x, dtype=np.float32), (0, 2, 1))).astype(ml_dtypes.bfloat16)
    m = np.asarray(mask).reshape(B, S).astype(np.float32)
    bq = np.asarray(b_qkv, np.float32)[:D]
    beff = (np.asarray(b_qkv, np.float64)[2 * D:] @ np.asarray(W_out, np.float64)
            + np.asarray(b_out, np.float64)).astype(np.float32)
    w = np.asarray(W_qkv, np.float32)
    # column permutation [q0|k0|q1..q5|k1..k5|v] so the kernel's first weight
    # DMA is one contiguous slice covering both chunk-0 stationaries
    wqkv = np.concatenate(
        [w[:, 0:P], w[:, D:D + P], w[:, P:D], w[:, D + P:2 * D],
         w[:, 2 * D:]], axis=1).astype(ml_dtypes.bfloat16)
    wout = np.asarray(W_out, np.float32).astype(ml_dtypes.bfloat16)
    # mask (cols 0:8) and q-bias (cols 8:14) fused into one [128,14] constant,
    # flattened t-major to match the kernel's (t p) -> p t view
    misc = np.concatenate([m.reshape(B, NKT, P).transpose(0, 2, 1),
                           np.broadcast_to(bq.reshape(KC, P).T[None],
                                           (B, P, KC))], axis=2)
    misc_flat = np.ascontiguousarray(misc.transpose(0, 2, 1)).reshape(B, -1)
    return [
        {"xt": xT[b], "msk": misc_flat[b], "wqkv": wqkv,
         "wout": wout, "beff": beff}
        for b in range(B)
    ]


def kernel(x, mask, W_qkv, b_qkv, W_out, b_out):
    nc = _build()
    maps = _in_maps(x, mask, W_qkv, b_qkv, W_out, b_out)
    res = run_bass_kernel_spmd(nc, maps, list(range(B))).results
    out = np.stack([np.asarray(res[b]["out"]).astype(np.float32)
                    for b in range(B)])
    return out


# revision 39
# speedup vs baseline: 1.2016x; 1.0014x over previous
"""BERT multi-head attention on 8 Trainium2 NeuronCores, data-parallel over batch.

Problem: x[8,1024,768] fp32, 12 heads, qkv + masked softmax attention + out proj.
Each core handles one batch element end-to-end; host gathers the 8 outputs.

Per-core layout strategy (S=1024, D=768, H=12, Dh=64):
  - all matmuls are bf16 x bf16 with f32 PSUM accumulation (same PE rate as
    f32r, half the DMA/SBUF): x and the weights are host-cast to bf16.
  - x is fed TRANSPOSED (xT [D,S]); q,k are produced transposed (qT/kT [D,S]);
    scores are computed transposed (scoresT [k,q], f32 psum, 2 heads packed
    per 128-row PE pass via tile_position).
  - ctx runs in [q-partition, Dh-free] orientation: the exp'd scores p [k,q]
    (bf16) are the matmul STATIONARY and v (bf16, mask folded in, plus a
    masked ones-column) is the moving operand, so every ctx matmul is a
    fully-utilized 128x128x65 tile and the softmax denominator rides along
    as output column 64. A PSUM bank holds one live accumulation group
    (start=True marks the whole 2KB zero-region), so each ctx tile's 8-kt
    accumulation runs as one back-to-back group; the 8 groups of q-half X
    are slotted into q-half X+1's score/exp stream (p tiles persist, 16 bufs).
  - softmax normalization is free: a DVE reciprocal of the denominator column
    plus per-partition tensor_scalar_mul during the PSUM->SBUF evacuation.
  - ctx[q, headA|headB] tiles are flipped to the [d-chunk, s] layout the
    output projection needs by ONE DMA xbar block-transpose per q-half
    ([128,512] -> 4x[128,128] blocks, 3D out AP; zero PE cost).
  - k-bias is dropped entirely: softmax over k is invariant to the per-q
    constant (q+bq)@bk, so only the q-bias is applied.
  - PE fill discipline: qk projection chunks and the output projection are
    chopped into ~512-cycle closures and pumped between attention matmuls so
    the PE never waits on the Activation engine's exp (1038ns/tile vs 643ns
    of attention matmuls per kt). The sweep is qh0-front so all six ctxT
    qh0 halves finish early and the s-tile 0..3 output projection overlaps
    the last three score sweeps.
  - PSUM: scores double-buffered (4 banks) + ctx accumulator (2 banks,
    [128, 8, 128] f32 so each 65-col group stays inside a bank) + a 2x1-bank
    fill pool shared by qk/v/out-proj = exactly 8 banks.
"""

import sys

import numpy as np

try:
    import concourse.bass as bass
except ImportError:  # pragma: no cover
    sys.path.insert(0, "/opt/trn_rl_repo")
    import concourse.bass as bass

from collections import deque
from contextlib import ExitStack

import concourse.tile as tile
from concourse import bacc, mybir
from concourse._compat import with_exitstack
from concourse.bass_utils import run_bass_kernel_spmd

F32 = mybir.dt.float32
BF16 = mybir.dt.bfloat16
EXP = mybir.ActivationFunctionType.Exp
COPY = mybir.ActivationFunctionType.Copy

B, S, D, H, DH, P = 8, 1024, 768, 12, 64, 128
KC = D // P          # 6 contraction chunks of 128
NKT = S // P         # 8 k-tiles of 128
NP = H // 2          # 6 head pairs
SCALE = 1.0 / np.sqrt(DH)


@with_exitstack
def _emit(ctx: ExitStack, tc, out, xT, wqkv, wout, beff, msk):
    nc = tc.nc

    const = ctx.enter_context(tc.tile_pool(name="const", bufs=1))
    persist = ctx.enter_context(tc.tile_pool(name="persist", bufs=1))
    p_pool = ctx.enter_context(tc.tile_pool(name="p", bufs=16))
    small = ctx.enter_context(tc.tile_pool(name="small", bufs=2))
    ctxn_pool = ctx.enter_context(tc.tile_pool(name="ctxn", bufs=2))
    osb_pool = ctx.enter_context(tc.tile_pool(name="osb", bufs=4))

    # ------------- inputs / constants -------------
    # DMA emission order == service order; the first score matmuls need
    # wq chunk 0, wq chunk 6 and x half 0, so those three go first.
    # wqkv arrives host-permuted as [q0|k0|q1..q5|k1..k5|v]: the first DMA
    # grabs both chunk-0 stationaries in one contiguous 256-col slice.
    wq_view = wqkv.rearrange("(c p) n -> p c n", p=P)   # [128, 6, 2304] bf16
    xT_view = xT.rearrange("(c p) s -> p c s", p=P)     # [128, 6, 1024] bf16
    wqk_sb = persist.tile([P, KC, 2 * D], BF16)         # permuted q|k layout
    xT_sb = persist.tile([P, KC, S], BF16)
    misc_sb = const.tile([P, NKT + KC], F32)            # mask cols 0:8, bq 8:14
    for h in range(3):  # chunk-pair pieces so the first matmuls can chase
        cs = slice(2 * h, 2 * h + 2)
        nc.sync.dma_start(wqk_sb[:, cs, 0:2 * P], wq_view[:, cs, 0:2 * P])
        nc.sync.dma_start(xT_sb[:, cs, 0:512], xT_view[:, cs, 0:512])
        if h == 0:
            nc.sync.dma_start(misc_sb[:], msk.rearrange("(t p) -> p t", p=P))
    wv_sb = persist.tile([P, KC, D], BF16)
    nc.sync.dma_start(wqk_sb[:, :, 7 * P:8 * P], wq_view[:, :, 7 * P:8 * P])
    nc.sync.dma_start(xT_sb[:, :, 512:1024], xT_view[:, :, 512:1024])
    nc.sync.dma_start(wv_sb[:, :, 0:384], wq_view[:, :, 2 * D:2 * D + 384])
    nc.sync.dma_start(wv_sb[:, :, 384:768], wq_view[:, :, 2 * D + 384:3 * D])
    nc.sync.dma_start(wqk_sb[:, :, 2 * P:7 * P], wq_view[:, :, 2 * P:7 * P])
    nc.sync.dma_start(wqk_sb[:, :, 8 * P:12 * P], wq_view[:, :, 8 * P:12 * P])
    wo_sb = persist.tile([P, KC, D], BF16)
    nc.sync.dma_start(wo_sb[:], wout.rearrange("(c p) n -> p c n", p=P))
    beff_bc = const.tile([P, D], F32)
    nc.sync.dma_start(beff_bc[:], beff.partition_broadcast(P))
    ones_sb = const.tile([P, 6], F32)
    nc.vector.memset(ones_sb[:], 1.0)

    _wq_off = {m: (0 if m == 0 else P if m == KC else
                   (1 + m) * P if m < KC else (m - KC + 6) * P)
               for m in range(2 * KC)}

    def wq_t(m, c):  # stationary slice for qk chunk m in the permuted layout
        o = _wq_off[m]
        return wqk_sb[:, c, o:o + P]

    qkT_sb = persist.tile([P, 2 * KC, S], BF16)    # chunks 0..5 = qT, 6..11 = kT
    v_sb = persist.tile([P, NKT, H, DH + 1], BF16)  # masked v + masked ones col
    ctxT0 = persist.tile([P, KC, 512], BF16)   # [d-chunk, s 0:512]
    ctxT1 = persist.tile([P, KC, 512], BF16)   # [d-chunk, s 512:1024]
    ctxT3d = [ctxT0[:].rearrange("p c (j q) -> p c j q", q=P),
              ctxT1[:].rearrange("p c (j q) -> p c j q", q=P)]

    def ctxT(c, qt):  # [128, 128] stationary block for s-tile qt
        t = ctxT0 if qt < 4 else ctxT1
        return t[:, c, (qt % 4) * P:(qt % 4 + 1) * P]

    # ------------- psum pools -------------
    ps_s = ctx.enter_context(tc.tile_pool(name="ps_s", bufs=2, space="PSUM"))
    ps_ctx = ctx.enter_context(tc.tile_pool(name="ps_ctx", bufs=1, space="PSUM"))
    ps_fill = ctx.enter_context(tc.tile_pool(name="ps_fill", bufs=2, space="PSUM"))

    # ------------- fill work: qk projection halves, out projection ----------
    # Each closure is ~512 PE cycles; pumped between attention matmuls so the
    # PE always has slack-independent work while ACT exps catch up.
    fill_q = deque()

    def pump(n):
        for _ in range(n):
            if not fill_q:
                return
            fill_q.popleft()()

    def qk_half_closures(m, n):
        # 6 accumulation matmuls into a 1-bank fill tile + a DVE evac
        st = {}

        def chunk(c):
            def go():
                if c == 0:
                    st["t"] = ps_fill.tile([P, 512], F32, tag="fill",
                                           name="qkfill")
                nc.tensor.matmul(
                    st["t"][:, 0:512],
                    wq_t(m, c),
                    xT_sb[:, c, n * 512:(n + 1) * 512],
                    start=(c == 0), stop=(c == KC - 1))
                if c == KC - 1:
                    dst = qkT_sb[:, m, n * 512:(n + 1) * 512]
                    if m < KC:  # q gets its bias; k-bias is softmax-invariant
                        nc.vector.tensor_scalar_add(dst, st["t"][:, 0:512],
                                                    misc_sb[:, NKT + m:NKT + m + 1])
                    else:
                        nc.vector.tensor_copy(dst, st["t"][:, 0:512])
            return go
        return [chunk(c) for c in range(KC)]

    # ----- V projection, one s-chunk, one half (6 heads), masked + ones col --
    def emit_v_st(st_i, half):
        pv = ps_fill.tile([P, 512], F32, tag="fill")
        for c in range(KC):
            nc.tensor.matmul(
                pv[:, 0:384],
                xT_sb[:, c, st_i * P:(st_i + 1) * P],
                wv_sb[:, c, half * 384:(half + 1) * 384],
                start=(c == 0), stop=(c == KC - 1))
        hs = slice(half * 6, (half + 1) * 6)
        nc.vector.tensor_scalar_mul(
            v_sb[:, st_i, hs, 0:DH],
            pv[:, 0:384].rearrange("p (h d) -> p h d", h=6),
            misc_sb[:, st_i:st_i + 1])
        nc.gpsimd.tensor_scalar_mul(
            v_sb[:, st_i, hs, DH:DH + 1],
            ones_sb[:].unsqueeze(2),
            misc_sb[:, st_i:st_i + 1])

    # ------------- out projection for one 128-row s-tile -------------
    def outproj_closures(qt):
        st = {}

        def mk(piece, c):
            def go():
                if c == 0:
                    st[piece] = ps_fill.tile([P, 512], F32, tag="fill",
                                             name="ofill")
                    if piece == 0:
                        st["o"] = osb_pool.tile([P, D], BF16, tag="o",
                                                name="osb")
                nc.tensor.matmul(
                    st[piece][:, 0:384],
                    ctxT(c, qt),
                    wo_sb[:, c, piece * 384:(piece + 1) * 384],
                    start=(c == 0), stop=(c == KC - 1))
                lo = piece * 384
                if c == KC - 1:
                    nc.vector.tensor_add(st["o"][:, lo:lo + 384],
                                         st[piece][:, 0:384],
                                         beff_bc[:, lo:lo + 384])
                    if piece == 1:
                        nc.sync.dma_start(out[qt * P:(qt + 1) * P, :],
                                          st["o"][:])
            return go
        order = ([(0, c) for c in range(KC - 1)]
                 + [(1, c) for c in range(KC - 1)]
                 + [(0, KC - 1), (1, KC - 1)])
        return [mk(piece, c) for piece, c in order]

    # ------------- attention, pipelined one q-half deep -------------
    def ctx_group(pair, ctx_t, p_list, slot, srcs=range(NKT)):
        # slot 0..7 -> groups A0,B0,A1,B1,... (banks alternate, so each bank
        # has at most one live accumulation group). srcs lets a group be
        # emitted in two pieces: kt_src 0..5 are ungated by the previous
        # item's last exps, 6..7 are not.
        j, half = slot // 2, slot % 2
        h = 2 * pair + half
        jj = 4 * half + j
        for kt in srcs:
            nc.tensor.matmul(
                ctx_t[:, jj, 0:DH + 1],
                p_list[kt][:, half * 512 + j * P:half * 512 + (j + 1) * P],
                v_sb[:, kt, h, :],
                start=(kt == 0), stop=(kt == NKT - 1),
                skip_group_check=True)

    def finish_qh(pair, qh, ctx_t, fast=False):
        # normalization + evacuation + transpose: one strided DVE reciprocal,
        # 8 evac-muls, two xbar half-transposes. fast=True splits the muls
        # across DVE and Pool for the tail-critical last q-half.
        r_sb = small.tile([P, NKT], F32, tag="r")
        nc.vector.reciprocal(r_sb[:].unsqueeze(2), ctx_t[:, :, DH:DH + 1])
        cn = ctxn_pool.tile([P, 512], BF16, tag="cn", name="cn")

        def mul(j, eng):
            if eng is nc.scalar:
                nc.scalar.activation(cn[:, j * P:j * P + DH],
                                     ctx_t[:, j, 0:DH], COPY,
                                     scale=r_sb[:, j:j + 1])
                nc.scalar.activation(cn[:, j * P + DH:(j + 1) * P],
                                     ctx_t[:, 4 + j, 0:DH], COPY,
                                     scale=r_sb[:, 4 + j:4 + j + 1])
                return
            eng.tensor_scalar_mul(cn[:, j * P:j * P + DH],
                                  ctx_t[:, j, 0:DH], r_sb[:, j:j + 1])
            eng.tensor_scalar_mul(cn[:, j * P + DH:(j + 1) * P],
                                  ctx_t[:, 4 + j, 0:DH],
                                  r_sb[:, 4 + j:4 + j + 1])

        e1 = nc.scalar if fast else nc.vector
        mul(0, nc.vector)
        mul(1, e1)
        nc.sync.dma_start(ctxT3d[qh][:, pair, 0:2, :],
                          cn[:, 0:256], transpose=True)
        mul(2, nc.vector)
        mul(3, e1)
        nc.sync.dma_start(ctxT3d[qh][:, pair, 2:4, :],
                          cn[:, 256:512], transpose=True)

    def attention(pair, qh, prev, v_half=None, ppk=2, pre_kt=None):
        # returns (pair, qh, p_list) to be consumed by the next call
        if isinstance(ppk, int):
            ppk = [ppk] * NKT
        pre_kt = pre_kt or {}
        qs = slice(qh * 512, (qh + 1) * 512)
        ctx_t = None
        p_list = []
        for kt in range(NKT):
            for fn in pre_kt.get(kt, ()):
                fn()
            if prev is not None and kt < 4:
                # ungated front halves of two ctx groups run BEFORE the
                # exp-gated kt0 scores, absorbing the item-boundary stall;
                # each group's last two kt_srcs trail one kt behind
                if ctx_t is None:
                    ctx_t = ps_ctx.tile([P, NKT, P], F32, tag="ctx")
                if kt > 0:
                    ctx_group(prev[0], ctx_t, prev[2], 2 * kt - 2,
                              srcs=range(6, NKT))
                    ctx_group(prev[0], ctx_t, prev[2], 2 * kt - 1,
                              srcs=range(6, NKT))
                ctx_group(prev[0], ctx_t, prev[2], 2 * kt, srcs=range(6))
                ctx_group(prev[0], ctx_t, prev[2], 2 * kt + 1, srcs=range(6))
            s_ps = ps_s.tile([P, 1024], F32, tag="s_ps")
            nc.tensor.matmul(
                s_ps[:, 0:512],
                qkT_sb[0:DH, KC + pair, kt * P:(kt + 1) * P],
                qkT_sb[0:DH, pair, qs],
                start=True, stop=True, tile_position=(0, 0))
            nc.tensor.matmul(
                s_ps[:, 512:1024],
                qkT_sb[DH:P, KC + pair, kt * P:(kt + 1) * P],
                qkT_sb[DH:P, pair, qs],
                start=True, stop=True, tile_position=(DH, 0))
            p_t = p_pool.tile([P, 1024], BF16)
            nc.scalar.activation(p_t[:], s_ps[:], EXP, bias=0.0, scale=SCALE)
            p_list.append(p_t)
            if v_half is not None and kt > 3:
                emit_v_st(kt - 4, v_half)
            if prev is not None and kt == 4:
                ctx_group(prev[0], ctx_t, prev[2], 6, srcs=range(6, NKT))
                ctx_group(prev[0], ctx_t, prev[2], 7, srcs=range(6, NKT))
                finish_qh(prev[0], prev[1], ctx_t)
            pump(ppk[kt])
        return (pair, qh, p_list)

    def drain_last(prev):
        ctx_t = ps_ctx.tile([P, NKT, P], F32, tag="ctx")
        for slot in range(NKT):
            pump(2)
            ctx_group(prev[0], ctx_t, prev[2], slot)
        finish_qh(prev[0], prev[1], ctx_t, fast=True)

    # ------------- phase structure -------------
    with nc.allow_low_precision(reason="bf16 stores are within the 2e-2 gate"):
        # pre-attention: q(0, half0) + k(6, half0), interleaved per chunk so
        # the matmuls chase the x DMA pieces
        tq = ps_fill.tile([P, 512], F32, tag="fill")
        tk = ps_fill.tile([P, 512], F32, tag="fill")
        for c in range(KC):
            nc.tensor.matmul(tq[:, 0:512], wq_t(0, c), xT_sb[:, c, 0:512],
                             start=(c == 0), stop=(c == KC - 1))
            nc.tensor.matmul(tk[:, 0:512], wq_t(KC, c), xT_sb[:, c, 0:512],
                             start=(c == 0), stop=(c == KC - 1))
        nc.vector.tensor_scalar_add(qkT_sb[:, 0, 0:512], tq[:, 0:512],
                                    misc_sb[:, NKT:NKT + 1])
        nc.vector.tensor_copy(qkT_sb[:, KC, 0:512], tk[:, 0:512])

        # sweep: qh0-front so every ctxT qh0 half is transposed by item 9 and
        # the s-tile 0..3 out projection pumps through items 10-12.
        # fill deadlines (emission order, in-order PE): a half must be fully
        # emitted before the first score matmul that reads it.
        k61 = qk_half_closures(KC, 1)   # k(6) half1: inline before kt4, needs
        #                                 x half1 which lands mid-item-1
        P18 = [1, 1, 1, 1, 2, 4, 4, 4]
        P12 = [0, 0, 1, 1, 2, 2, 3, 3]
        P16 = [0, 1, 1, 1, 2, 3, 4, 4]
        sweep = [
            # (pair, qh, v_half, pumped_halves, ppk)
            (0, 0, 0, [(KC + 1, 0)], [2, 2, 1, 1, 0, 0, 0, 0]),
            (0, 1, None, [(KC + 1, 1), (1, 0), (KC + 2, 0)], P18),
            (1, 0, 1, [], 0),
            (1, 1, None, [(KC + 2, 1), (2, 0), (KC + 3, 0)], P18),
            (2, 0, None, [(KC + 3, 1), (3, 0), (KC + 4, 0)], P18),
            (3, 0, None, [(KC + 4, 1), (4, 0), (KC + 5, 0)], P18),
            (4, 0, None, [(KC + 5, 1), (5, 0)], P12),
            (5, 0, None, [(2, 1), (3, 1)], P12),
            (2, 1, None, [(4, 1), (5, 1)], P12),
            (3, 1, "op", [], P16),
            (4, 1, None, [], P16),
            (5, 1, None, [], P16),
        ]
        posts = {(0, 0): [(0, 1)], (1, 0): [(1, 1)]}
        prev = None
        for i, (pair, qh, vh, pumped, ppk) in enumerate(sweep):
            if vh == "op":
                vh = None
                for qt in range(4):
                    fill_q.extend(outproj_closures(qt))
            for key in pumped:
                fill_q.extend(qk_half_closures(*key))
            pre = {}
            if i == 0:
                # k(6,1) emitted compactly before kt4 (its x-half-1 input
                # lands mid-item); v s-chunks run four kts late (the wv and
                # x DMAs land mid-item), spilling into the next item's pre
                pre = {4: k61}
            if i in (1, 3):
                vh_prev = 0 if i == 1 else 1
                pre = {0: [lambda h=vh_prev: emit_v_st(4, h),
                           lambda h=vh_prev: emit_v_st(5, h)],
                       1: [lambda h=vh_prev: emit_v_st(6, h),
                           lambda h=vh_prev: emit_v_st(7, h)]}
            prev = attention(pair, qh, prev, v_half=vh, ppk=ppk, pre_kt=pre)
            for key in posts.get((pair, qh), ()):
                for fn in qk_half_closures(*key):
                    fn()
        t4 = ps_s.tile([P, 1024], F32, tag="s_ps", name="ost")
        head = [(4, 0, t4[:, 0:384]), (4, 1, t4[:, 512:896])]
        for qt, piece, ap in head:
            for c in range(KC - 1):
                nc.tensor.matmul(ap, ctxT(c, qt),
                                 wo_sb[:, c, piece * 384:(piece + 1) * 384],
                                 start=(c == 0), stop=(c == KC - 2))
            nc.vector.tensor_add(ap, ap, beff_bc[:, piece * 384:
                                                 piece * 384 + 384])
        drain_last(prev)
        pump(len(fill_q))
        pieces = list(head)   # (qt, piece, psum ap)
        for qt in (5,):
            t = ps_s.tile([P, 1024], F32, tag="s_ps", name="ost")
            pieces += [(qt, 0, t[:, 0:384]), (qt, 1, t[:, 512:896])]
        tc6 = ps_ctx.tile([P, NKT, P], F32, tag="ctx", name="oct")
        t6 = tc6[:].rearrange("p j q -> p (j q)")
        pieces += [(6, 0, t6[:, 0:384]), (6, 1, t6[:, 512:896])]
        for qt in (7,):
            ta = ps_fill.tile([P, 512], F32, tag="fill", name="ofa")
            tb = ps_fill.tile([P, 512], F32, tag="fill", name="ofb")
            pieces += [(qt, 0, ta[:, 0:384]), (qt, 1, tb[:, 0:384])]
        for qt, piece, ap in pieces[2:]:    # pairs 0..4 stream immediately
            for c in range(KC - 1):
                nc.tensor.matmul(ap, ctxT(c, qt),
                                 wo_sb[:, c, piece * 384:(piece + 1) * 384],
                                 start=(c == 0), stop=(c == KC - 2))
            nc.vector.tensor_add(ap, ap, beff_bc[:, piece * 384:
                                                 piece * 384 + 384])
        osbs = {}
        for qt, piece, ap in pieces:        # pair-5 chunk gated on transpose
            nc.tensor.matmul(ap, ctxT(KC - 1, qt),
                             wo_sb[:, KC - 1, piece * 384:(piece + 1) * 384],
                             start=False, stop=True, skip_group_check=True)
            if piece == 0:
                osbs[qt] = osb_pool.tile([P, D], BF16, tag="o", name="osb")
            lo = piece * 384
            if piece == 0:
                nc.vector.tensor_copy(osbs[qt][:, lo:lo + 384], ap)
            else:
                nc.scalar.activation(osbs[qt][:, lo:lo + 384], ap, COPY)
            if piece == 1:
                nc.sync.dma_start(out[qt * P:(qt + 1) * P, :], osbs[qt][:])


_CACHE = {}


def _build():
    if "nc" in _CACHE:
        return _CACHE["nc"]
    nc = bacc.Bacc("TRN2", target_bir_lowering=False, debug=False,
                   num_devices=B)
    xT = nc.dram_tensor("xt", [D, S], BF16, kind="ExternalInput").ap()
    wqkv = nc.dram_tensor("wqkv", [D, 3 * D], BF16, kind="ExternalInput").ap()
    wout = nc.dram_tensor("wout", [D, D], BF16, kind="ExternalInput").ap()
    beff = nc.dram_tensor("beff", [D], F32, kind="ExternalInput").ap()
    msk = nc.dram_tensor("msk", [P * (NKT + KC)], F32,
                         kind="ExternalInput").ap()
    out = nc.dram_tensor("out", [S, D], BF16, kind="ExternalOutput").ap()
    with tile.TileContext(nc) as tc:
        _emit(tc, out, xT, wqkv, wout, beff, msk)
    nc.compile()
    _CACHE["nc"] = nc
    return nc


def _in_maps(x, mask, W_qkv, b_qkv, W_out, b_out):
    import ml_dtypes
    xT = np.ascontiguousarray(np.transpose(
        np.asarray(x, dtype=np.float32), (0, 2, 1))).astype(ml_dtypes.bfloat16)
    m = np.asarray(mask).reshape(B, S).astype(np.float32)
    bq = np.asarray(b_qkv, np.float32)[:D]
    beff = (np.asarray(b_qkv, np.float64)[2 * D:] @ np.asarray(W_out, np.float64)
            + np.asarray(b_out, np.float64)).astype(np.float32)
    w = np.asarray(W_qkv, np.float32)
    # column permutation [q0|k0|q1..q5|k1..k5|v] so the kernel's first weight
    # DMA is one contiguous slice covering both chunk-0 stationaries
    wqkv = np.concatenate(
        [w[:, 0:P], w[:, D:D + P], w[:, P:D], w[:, D + P:2 * D],
         w[:, 2 * D:]], axis=1).astype(ml_dtypes.bfloat16)
    wout = np.asarray(W_out, np.float32).astype(ml_dtypes.bfloat16)
    # mask (cols 0:8) and q-bias (cols 8:14) fused into one [128,14] constant,
    # flattened t-major to match the kernel's (t p) -> p t view
    misc = np.concatenate([m.reshape(B, NKT, P).transpose(0, 2, 1),
                           np.broadcast_to(bq.reshape(KC, P).T[None],
                                           (B, P, KC))], axis=2)
    misc_flat = np.ascontiguousarray(misc.transpose(0, 2, 1)).reshape(B, -1)
    return [
        {"xt": xT[b], "msk": misc_flat[b], "wqkv": wqkv,
         "wout": wout, "beff": beff}
        for b in range(B)
    ]


def kernel(x, mask, W_qkv, b_qkv, W_out, b_out):
    nc = _build()
    maps = _in_maps(x, mask, W_qkv, b_qkv, W_out, b_out)
    res = run_bass_kernel_spmd(nc, maps, list(range(B))).results
    out = np.stack([np.asarray(res[b]["out"]).astype(np.float32)
                    for b in range(B)])
    return out
